# revision 1
# baseline (speedup 1.0000x reference)
"""DMPNN encoder on 8 Trainium2 NeuronCores (Bass/Tile).

Strategy (data-parallel over graphs):
- Partition graphs into 8 contiguous chunks with ~equal atom counts
  (graph-aligned).  Each core owns the edges whose *target* atom lives in
  its chunk, sorted by local target -> segment-sum over targets is local.
- Per message-passing round, each core computes nei = segsum(msg) via
  selection-matrix matmuls, then Z = nei @ W_h on its own atoms, then the
  Z shards are AllGather'd across the 8 cores.  msg' = relu(msg + Z[src])
  only needs row gathers (indirect DMA) from the gathered Z table.
- Final round: atom_msg -> atom_h = relu([x||atom_msg] @ W_o + b_o) and
  sum-pool to graphs via selection matmuls; host sums partial group blocks.

All index manipulation is host-precomputed; the device does only dense
matmuls, elementwise ops, contiguous DMA and indirect row gathers.
"""

import os
import sys

for _p in ("/opt/trn_rl_repo", "/root/.axon_site/_ro/trn_rl_repo"):
    if os.path.isdir(_p) and _p not in sys.path:
        sys.path.insert(0, _p)

from contextlib import ExitStack

import numpy as np

import concourse.bass as bass
import concourse.tile as tile
from concourse import bacc, mybir
from concourse.bass_utils import run_bass_kernel_spmd
from concourse.masks import make_identity
from concourse.tile_rust import add_dep_helper

C = 8
H = 300
AF = 133
BF = 14
DEPTH = 3
NUM_GRAPHS = 4096

F32 = mybir.dt.float32
BF16 = mybir.dt.bfloat16
I32 = mybir.dt.int32

Relu = mybir.ActivationFunctionType.Relu
Copy = mybir.ActivationFunctionType.Copy
ADD = mybir.AluOpType.add
ISEQ = mybir.AluOpType.is_equal
BYPASS = mybir.AluOpType.bypass

IOA = bass.IndirectOffsetOnAxis


def ceil_to(x, m):
    return ((x + m - 1) // m) * m


# ---------------------------------------------------------------------------
# host-side preprocessing
# ---------------------------------------------------------------------------

def preprocess(x, edge_attr, edge_index, batch, num_graphs):
    N = x.shape[0]
    src = edge_index[0].astype(np.int64)
    tgt = edge_index[1].astype(np.int64)
    batch = batch.astype(np.int64)

    graph_start = np.searchsorted(batch, np.arange(num_graphs + 1))
    targets = (np.arange(1, C) * N) // C
    split_graphs = np.searchsorted(graph_start, targets)
    atom_splits = [0] + [int(graph_start[g]) for g in split_graphs] + [N]
    a0 = np.array(atom_splits[:-1])
    a1 = np.array(atom_splits[1:])
    n_real = a1 - a0
    N_pad = ceil_to(int(n_real.max()), 128) + 128
    NA = N_pad // 128

    owner = np.zeros(N, dtype=np.int64)
    loc = np.zeros(N, dtype=np.int64)
    for c in range(C):
        owner[a0[c]:a1[c]] = c
        loc[a0[c]:a1[c]] = np.arange(n_real[c])

    e_owner = owner[tgt]

    # per-core, per-ablock sorted edge lists (slot order within an ablock is
    # irrelevant for the selection matmuls)
    ab_eids = []  # [c][a] -> array of edge ids
    counts = np.zeros((C, NA), dtype=np.int64)
    for c in range(C):
        eids = np.nonzero(e_owner == c)[0]
        order = np.argsort(loc[tgt[eids]], kind="stable")
        eids = eids[order]
        ab = loc[tgt[eids]] // 128
        cuts = np.searchsorted(ab, np.arange(NA + 1))
        ab_eids.append([eids[cuts[a]:cuts[a + 1]] for a in range(NA)])
        counts[c] = cuts[1:] - cuts[:-1]

    for GRP in (64, 32, 16, 8, 4, 2, 1):
        ok = True
        for c in range(C):
            nb = int(n_real[c])
            for g0a in range(0, NA, GRP):
                alo, ahi = g0a * 128, min((g0a + GRP) * 128, nb)
                if alo >= nb:
                    continue
                if batch[a0[c] + ahi - 1] - batch[a0[c] + alo] >= 128:
                    ok = False
                    break
            if not ok:
                break
        if ok:
            break
    NGRP = (NA + GRP - 1) // GRP

    # greedy segmentation over ablocks: keep the union live-depth of nei
    # accumulators <= DMAX so only DMAX PSUM slots are needed
    DMAX = 3

    def seg_stats(s, e):
        P = np.zeros((C,), dtype=np.int64)
        first = {}
        last = {}
        for a in range(s, e):
            cnt = counts[:, a]
            act = cnt > 0
            if act.any():
                first[a] = int((P[act] // 128).min())
                last[a] = int(((P[act] + cnt[act] - 1) // 128).max())
            P += cnt
        ntile = int(-(-P.max() // 128)) if P.max() else 1
        depth = 0
        for t in range(ntile):
            depth = max(depth, sum(1 for a in first
                                   if first[a] <= t <= last[a]))
        return first, last, ntile, depth

    segments = []  # (a_start, a_end, first, last, ntiles)
    s = 0
    while s < NA:
        e = s + 1
        stats = seg_stats(s, e)
        while e < NA and (e % GRP) != 0:
            cand = seg_stats(s, e + 1)
            if cand[3] > DMAX:
                break
            e += 1
            stats = cand
        segments.append((s, e, stats[0], stats[1], stats[2]))
        s = e

    T = sum(sg[4] for sg in segments)
    Epad = T * 128

    contribs = [[] for _ in range(T)]
    base = 0
    for (s, e, first, last, ntiles) in segments:
        for a in range(s, e):
            if a in first:
                ts = range(base + first[a], base + last[a] + 1)
            else:
                ts = [base]  # no edges anywhere: zero matmul, still drains
            ts = list(ts)
            for i, t in enumerate(ts):
                contribs[t].append((a, i == 0, i == len(ts) - 1))
        base += ntiles

    # contrib enumeration order + chunk boundaries for batched sel builds
    contrib_idx = {}
    ci = 0
    for t in range(T):
        for (a, f, l) in contribs[t]:
            contrib_idx[(t, a)] = ci
            ci += 1
    NCONTRIB = ci

    drain_seq = [a for t in range(T) for (a, f, l) in contribs[t] if l]
    pool_first = {}
    pool_last = {}
    seen = {}
    for a in drain_seq:
        j = a // GRP
        if j not in seen:
            pool_first[a] = True
        seen[j] = a
    for j, a in seen.items():
        pool_last[a] = True

    # collective chunking: chunk-major Z-table layout so each chunk's
    # AllGather output is a contiguous region.  Sizes decrease so the last
    # chunk's exchange latency (the only un-overlappable part) is tiny.
    gu = (NA + GRP - 1) // GRP  # group units
    unit_counts = []
    rem = gu
    for frac in (3, 3, 4, 6, 10, 16):
        take = max(1, round(rem_part) if (rem_part := gu / frac) and False else
                   max(1, gu // frac))
        take = min(take, rem)
        if take:
            unit_counts.append(take)
            rem -= take
        if rem == 0:
            break
    while rem > 0:
        unit_counts.append(1)
        rem -= 1
    cc_chunks = []
    a = 0
    for u in unit_counts:
        e = min(a + u * GRP, NA)
        cc_chunks.append((a, e))
        a = e
        if a >= NA:
            break
    n_cc = len(cc_chunks)
    cc_start = np.array([k0 * 128 for (k0, k1) in cc_chunks])
    cc_rows = np.array([(k1 - k0) * 128 for (k0, k1) in cc_chunks])
    cc_off = np.concatenate([[0], np.cumsum(C * cc_rows)])[:-1]

    # lay out slots: per core, segments concatenated, padded to common size
    per_core = []
    for c in range(C):
        slot_eid = np.full(Epad, -1, dtype=np.int64)
        base = 0
        for (s, e, first, last, ntiles) in segments:
            seg = np.concatenate([ab_eids[c][a] for a in range(s, e)]) \
                if e > s else np.zeros(0, np.int64)
            slot_eid[base:base + len(seg)] = seg
            base += ntiles * 128
        real = slot_eid >= 0
        eids = slot_eid[real]
        ne = int(real.sum())
        tloc = np.full(Epad, -1, dtype=np.int64)
        tloc[real] = loc[tgt[eids]]
        src_g = np.zeros(Epad, dtype=np.int64)
        src_g[real] = src[eids]
        ea = np.zeros((Epad, BF), dtype=np.float32)
        ea[real] = edge_attr[eids]
        zrow = np.zeros(Epad, dtype=np.int64)
        ol = owner[src_g[real]]
        ll = loc[src_g[real]]
        ck = np.searchsorted(cc_start, ll, side="right") - 1
        zrow[real] = cc_off[ck] + ol * cc_rows[ck] + (ll - cc_start[ck])
        uniq, inv = np.unique(src_g[real], return_inverse=True)
        xidx = np.zeros(Epad, dtype=np.int64)
        xidx[real] = inv
        ctab = np.zeros(NCONTRIB, dtype=object)
        ctabm = np.zeros((128, NCONTRIB), dtype=np.float32)
        for t in range(T):
            for (a, f, l) in contribs[t]:
                i = contrib_idx[(t, a)]
                ctabm[:, i] = (tloc[t * 128:(t + 1) * 128] - 128 * a)
        per_core.append(dict(ne=ne, real=real, tloc=tloc, ea=ea, zrow=zrow,
                             xidx=xidx, uniq=uniq, n_uniq=len(uniq),
                             ctab=ctabm))

    X_pad = max(pc["n_uniq"] for pc in per_core) + 1
    for pc in per_core:
        pc["xidx"][~pc["real"]] = X_pad - 1
        xs = np.zeros((X_pad, AF), dtype=np.float32)
        xs[: pc["n_uniq"]] = x[pc["uniq"]]
        pc["x_sub"] = xs

    for c in range(C):
        gloc = np.full(N_pad, 1 << 20, dtype=np.int64)
        g0s = np.full(NGRP, -1, dtype=np.int64)
        nb = int(n_real[c])
        for j in range(NGRP):
            alo = j * GRP * 128
            ahi = min(alo + GRP * 128, nb)
            if alo >= nb:
                continue
            g0 = batch[a0[c] + alo]
            g0s[j] = g0
            gloc[alo:ahi] = batch[a0[c] + alo: a0[c] + ahi] - g0
        per_core[c]["gloc"] = gloc
        per_core[c]["g0s"] = g0s
        xo = np.zeros((N_pad, AF), dtype=np.float32)
        xo[:nb] = x[a0[c]: a1[c]]
        per_core[c]["x_own"] = xo

    meta = dict(N_pad=N_pad, NA=NA, Epad=Epad, T=T, X_pad=X_pad, GRP=GRP,
                NGRP=NGRP, a0=a0, a1=a1, n_real=n_real, contribs=contribs,
                pool_first=pool_first, pool_last=pool_last,
                contrib_idx=contrib_idx, NCONTRIB=NCONTRIB,
                cc_chunks=cc_chunks, cc_off=cc_off, cc_rows=cc_rows)
    return per_core, meta


# ---------------------------------------------------------------------------
# the Bass program (identical for all 8 cores; data differs per core)
# ---------------------------------------------------------------------------

def build_program(meta, DT=F32, CH=4):
    T, NA, N_pad = meta["T"], meta["NA"], meta["N_pad"]
    Epad, X_pad = meta["Epad"], meta["X_pad"]
    GRP, NGRP = meta["GRP"], meta["NGRP"]
    contribs = meta["contribs"]

    nc = bacc.Bacc("TRN2", target_bir_lowering=False, debug=False,
                   num_devices=C)

    X_pad2 = ceil_to(X_pad, 128)
    xsubT0 = nc.dram_tensor("xsubT0", [128, X_pad2], DT, kind="ExternalInput")
    xsubT1 = nc.dram_tensor("xsubT1", [AF - 128, X_pad2], DT,
                            kind="ExternalInput")
    xownT0 = nc.dram_tensor("xownT0", [128, N_pad], DT, kind="ExternalInput")
    xownT1 = nc.dram_tensor("xownT1", [AF - 128, N_pad], DT,
                            kind="ExternalInput")
    ea_in = nc.dram_tensor("ea_in", [BF, Epad], DT, kind="ExternalInput")
    ctab_in = nc.dram_tensor("ctab_in", [128, meta["NCONTRIB"]], F32,
                             kind="ExternalInput")
    zrow_in = nc.dram_tensor("zrow_in", [128, T], I32, kind="ExternalInput")
    xidx_in = nc.dram_tensor("xidx_in", [128, T], I32, kind="ExternalInput")
    gloc_in = nc.dram_tensor("gloc_in", [128, NA], F32, kind="ExternalInput")
    iota_in = nc.dram_tensor("iota_in", [128, 128], F32, kind="ExternalInput")
    bo_in = nc.dram_tensor("bo_in", [128, H], F32, kind="ExternalInput")

    wname_shapes = dict(
        wix0=[128, H], wix1=[AF - 128, H], wie=[BF, H],
        wh0=[128, H], wh1=[128, H], wh2=[H - 256, H],
        wox0=[128, H], wox1=[AF - 128, H],
        wom0=[128, H], wom1=[128, H], wom2=[H - 256, H],
    )
    w_in = {k: nc.dram_tensor(k, s, DT, kind="ExternalInput")
            for k, s in wname_shapes.items()}

    molp = nc.dram_tensor("molp", [NGRP * 128, H], F32, kind="ExternalOutput")

    zfull1 = nc.dram_tensor("zfull1", [C * N_pad, H], DT, addr_space="Shared")
    zfull2 = nc.dram_tensor("zfull2", [C * N_pad, H], DT, addr_space="Shared")
    cc_chunks = meta["cc_chunks"]

    HC = [(0, 128), (128, 256), (256, H)]  # hidden-dim K chunks
    XC = [(0, 128), (128, AF)]             # atom-feature K chunks

    with tile.TileContext(nc) as tc, ExitStack() as ctx:
        const = ctx.enter_context(tc.tile_pool(name="const", bufs=1))
        sb = ctx.enter_context(tc.tile_pool(name="sb", bufs=4))
        ps = ctx.enter_context(tc.tile_pool(name="ps", bufs=2, space="PSUM"))
        dram = ctx.enter_context(tc.tile_pool(name="dram", bufs=1,
                                              space="DRAM"))

        # ---- residents ----
        def cload(name, src, shape, dt):
            tl = const.tile(shape, dt, tag=name)
            nc.sync.dma_start(tl[:], src[:])
            return tl

        ctab_s = cload("ctab", ctab_in, [128, meta["NCONTRIB"]], F32)
        zrow_s = cload("zrow", zrow_in, [128, T], I32)
        xidx_s = cload("xidx", xidx_in, [128, T], I32)
        gloc_s = cload("gloc", gloc_in, [128, NA], F32)
        iota_s = cload("iota", iota_in, [128, 128], F32)
        bo_s = cload("bo", bo_in, [128, H], F32)
        w_s = {k: cload(k, w_in[k], wname_shapes[k], DT) for k in w_in}
        identF = const.tile([128, 128], F32, tag="identF")
        make_identity(nc, identF[:])


        msg1 = dram.tile([128, T * H], DT, tag="msg1")
        msg2 = dram.tile([128, T * H], DT, tag="msg2")
        zsh1 = dram.tile([N_pad, H], DT, tag="zsh1")
        zsh2 = dram.tile([N_pad, H], DT, tag="zsh2")

        psum_nei = {}
        psum_pool = {}

        def transpose_chunks(src_ap, chunks, ident):
            """PE-transpose column chunks of src_ap -> list of SBUF DT tiles."""
            out = []
            for (c0, c1) in chunks:
                w = c1 - c0
                tp = ps.tile([128, 128], F32, tag="pB", bufs=2)
                nc.tensor.transpose(tp[:w, :], src_ap[:, c0:c1], ident[:])
                trs = sb.tile([128, 128], DT, tag="trs")
                nc.vector.tensor_copy(trs[:w, :], tp[:w, :])
                out.append(trs)
            return out

        cc_state = {}

        cur_zfull = [None]

        def drain_z(a, zsh):
            """psum_nei[a] -> Z = nei @ W_h -> zsh rows of ablock a."""
            nei_sb = sb.tile([128, H], F32, tag="neisb")
            nc.vector.tensor_copy(nei_sb[:], psum_nei.pop(a)[:])
            trs = transpose_chunks(nei_sb, HC, identF)
            zps = ps.tile([128, H], F32, tag="pD", bufs=1)
            for ci, ((c0, c1), wt) in enumerate(
                    zip(HC, (w_s["wh0"], w_s["wh1"], w_s["wh2"]))):
                w = c1 - c0
                nc.tensor.matmul(zps[:], lhsT=trs[ci][:w, :], rhs=wt[:],
                                 start=(ci == 0), stop=(ci == len(HC) - 1))
            zsb = sb.tile([128, H], DT, tag="zsb")
            nc.scalar.copy(zsb[:], zps[:])
            nc.sync.dma_start(zsh[a * 128:(a + 1) * 128, :], zsb[:])
            fire_cc(a, zsh, cur_zfull[0])

        def drain_final(a):
            """psum_nei[a] = atom_msg -> atom_h -> pool into group psum."""
            am_sb = sb.tile([128, H], F32, tag="neisb")
            nc.vector.tensor_copy(am_sb[:], psum_nei.pop(a)[:])
            hps = ps.tile([128, H], F32, tag="pA", bufs=2)
            at = transpose_chunks(am_sb, HC, identF)
            asl = slice(a * 128, (a + 1) * 128)
            xo0 = sb.tile([128, 128], DT, tag="xo0")
            nc.sync.dma_start(xo0[:], xownT0[:, asl])
            xo1 = sb.tile([AF - 128, 128], DT, tag="xo1")
            nc.sync.dma_start(xo1[:], xownT1[:, asl])
            parts = [(XC[0], xo0[:], w_s["wox0"]),
                     (XC[1], xo1[:], w_s["wox1"])] + \
                [((c0, c1), at[ci][:c1 - c0, :], w_s[k])
                 for ci, ((c0, c1), k) in enumerate(
                     zip(HC, ("wom0", "wom1", "wom2")))]
            for ci, ((c0, c1), lh, wt) in enumerate(parts):
                nc.tensor.matmul(hps[:], lhsT=lh, rhs=wt[:],
                                 start=(ci == 0), stop=(ci == len(parts) - 1))
            hsum = sb.tile([128, H], DT, tag="hsum")
            nc.vector.tensor_tensor(hsum[:], hps[:], bo_s[:], op=ADD)
            hrelu = sb.tile([128, H], DT, tag="hrelu")
            nc.scalar.activation(hrelu[:], hsum[:], Relu)
            selp = sb.tile([128, 128], DT, tag="selp")
            nc.vector.tensor_tensor(
                selp[:], gloc_s[:, a:a + 1].to_broadcast([128, 128]),
                iota_s[:], op=ISEQ)
            j = a // GRP
            first = meta["pool_first"].get(a, False)
            last = meta["pool_last"].get(a, False)
            if first:
                psum_pool[j] = ps.tile([128, H], F32, tag="pD", name=f"pool{j}", bufs=1)
            nc.tensor.matmul(psum_pool[j][:], lhsT=selp[:], rhs=hrelu[:],
                             start=first, stop=last)
            if last:
                mol_sb = sb.tile([128, H], F32, tag="molsb")
                nc.vector.tensor_copy(mol_sb[:], psum_pool.pop(j)[:])
                nc.sync.dma_start(molp[j * 128:(j + 1) * 128, :], mol_sb[:])

        contrib_idx = meta["contrib_idx"]
        max_ncc = max(sum(len(contribs[t]) for t in range(t0, min(t0 + CH, T)))
                      for t0 in range(0, T, CH))

        def build_sels(t0, k):
            """One DVE op building all sel matrices for tiles [t0, t0+k)."""
            idxs = [contrib_idx[(t, a)] for t in range(t0, t0 + k)
                    for (a, f, l) in contribs[t]]
            if not idxs:
                return None, 0
            i0, ncc = idxs[0], len(idxs)
            assert idxs == list(range(i0, i0 + ncc))
            selc = sb.tile([128, ncc * 128], DT, tag="sel",
                           padded_shape=[128, max_ncc * 128], name="selc")
            base = ctab_s[:, i0:i0 + ncc]
            in0 = bass.AP(tensor=base.tensor, offset=base.offset,
                          ap=list(base.ap) + [[0, 128]])
            it = iota_s[:]
            in1 = bass.AP(tensor=it.tensor, offset=it.offset,
                          ap=[it.ap[0], [0, ncc], it.ap[1]])
            out3 = selc.rearrange("p (a b) -> p a b", b=128)
            nc.vector.tensor_tensor(out3, in0, in1, op=ISEQ)
            return selc, i0

        def segsum_contrib(msg_ap, t, rnd, zsh, selc, i0):
            for (a, first, last) in contribs[t]:
                q = contrib_idx[(t, a)] - i0
                sel_ap = selc[:, q * 128:(q + 1) * 128]
                if first:
                    psum_nei[a] = ps.tile([128, H], F32, tag="pC", name=f"nei{a}", bufs=3)
                nc.tensor.matmul(psum_nei[a][:], lhsT=sel_ap, rhs=msg_ap,
                                 start=first, stop=last)
                if last:
                    if rnd < DEPTH:
                        drain_z(a, zsh)
                    else:
                        drain_final(a)

        # ---- stage A: initial messages + round-1 segsum ----
        def fire_cc(a, zsh, zfull):
            """After ablock a's drain DMA: if it completes a cc chunk, fire
            that chunk's AllGather."""
            st = cc_state.setdefault(id(zsh), dict(done=set()))
            st["done"].add(a)
            for (k0, k1) in cc_chunks:
                if all(x in st["done"] for x in range(k0, k1)) \
                        and (k0, k1) not in st.get("fired", set()):
                    st.setdefault("fired", set()).add((k0, k1))
                    ci = cc_chunks.index((k0, k1))
                    off = int(meta["cc_off"][ci])
                    nrows = int(meta["cc_rows"][ci])
                    cc = nc.gpsimd.collective_compute(
                        "AllGather", BYPASS,
                        replica_groups=[list(range(C))],
                        ins=[zsh[k0 * 128:k1 * 128, :]],
                        outs=[zfull[off:off + C * nrows, :]])
                    st["last_cc"] = cc

        # ---- stage A1: XW = x_sub @ W_i[:AF] over the unique-row table ----
        last_xw_store = [None]
        xw = dram.tile([X_pad2, H], DT, tag="xw")
        XT = X_pad2 // 128
        for t0 in range(0, XT, CH):
            k = min(CH, XT - t0)
            csl = slice(t0 * 128, (t0 + k) * 128)
            xs0 = sb.tile([128, k * 128], DT, tag="xs0")
            nc.sync.dma_start(xs0[:], xsubT0[:, csl])
            xs1 = sb.tile([AF - 128, k * 128], DT, tag="xs1")
            nc.sync.dma_start(xs1[:], xsubT1[:, csl])
            xw_sb = sb.tile([128, k * H], DT, tag="xwsb", bufs=3)
            for j in range(k):
                jsl = slice(j * 128, (j + 1) * 128)
                xps = ps.tile([128, H], F32, tag="pA", bufs=2)
                nc.tensor.matmul(xps[:], lhsT=xs0[:, jsl],
                                 rhs=w_s["wix0"][:], start=True, stop=False)
                nc.tensor.matmul(xps[:], lhsT=xs1[:, jsl],
                                 rhs=w_s["wix1"][:], start=False, stop=True)
                nc.scalar.copy(xw_sb[:, j * H:(j + 1) * H], xps[:])
            xw_view = xw[t0 * 128:(t0 + k) * 128, :].rearrange(
                "(t p) h -> p t h", p=128)
            last_xw_store[0] = nc.sync.dma_start(xw_view, xw_sb[:])

        tc.strict_bb_all_engine_barrier()

        # ---- stage A2: msg1 = relu(XW[src] + ea @ W_i[AF:]) + segsum ----
        cur_zfull[0] = zfull1
        for t0 in range(0, T, CH):
            k = min(CH, T - t0)
            selc, i0c = build_sels(t0, k)
            ea_t = sb.tile([BF, k * 128], DT, tag="ea")
            nc.sync.dma_start(ea_t[:, :], ea_in[:, t0 * 128:(t0 + k) * 128])
            msg_sb = sb.tile([128, k * H], DT, tag="msg", bufs=3)
            for j in range(k):
                t = t0 + j
                xwg = sb.tile([128, H], DT, tag="xwg", bufs=6)
                nc.gpsimd.indirect_dma_start(
                    out=xwg[:], out_offset=None, in_=xw[:],
                    in_offset=IOA(ap=xidx_s[:, t:t + 1], axis=0))
                mps = ps.tile([128, H], F32, tag="pA", bufs=2)
                nc.tensor.matmul(mps[:], lhsT=ea_t[:, j * 128:(j + 1) * 128],
                                 rhs=w_s["wie"][:], start=True, stop=True)
                msum = sb.tile([128, H], DT, tag="msum")
                nc.vector.tensor_tensor(msum[:], mps[:], xwg[:], op=ADD)
                m_ap = msg_sb[:, j * H:(j + 1) * H]
                nc.scalar.activation(m_ap, msum[:], Relu)
                segsum_contrib(m_ap, t, 1, zsh1, selc, i0c)
            nc.sync.dma_start(msg1[:, t0 * H:(t0 + k) * H], msg_sb[:])

        # ---- stages B (round 2) and C (round 3 + readout) ----
        def stage_mid(msg_in, msg_out, zfull, zsh, rnd, cc_prev):
            zflat = zfull[:]
            for t0 in range(0, T, CH):
                k = min(CH, T - t0)
                selc, i0c = build_sels(t0, k)
                ld = sb.tile([128, k * H], DT, tag="ld", bufs=3)
                nc.sync.dma_start(ld[:], msg_in[:, t0 * H:(t0 + k) * H])
                mrel = sb.tile([128, k * H], DT, tag="msg", bufs=3)
                for j in range(k):
                    t = t0 + j
                    zg = sb.tile([128, H], DT, tag="zg", bufs=6)
                    nc.gpsimd.indirect_dma_start(
                        out=zg[:], out_offset=None, in_=zflat,
                        in_offset=IOA(ap=zrow_s[:, t:t + 1], axis=0))
                    msum = sb.tile([128, H], DT, tag="msum")
                    nc.vector.tensor_tensor(msum[:], ld[:, j * H:(j + 1) * H],
                                            zg[:], op=ADD)
                    nc.scalar.activation(mrel[:, j * H:(j + 1) * H], msum[:],
                                         Relu)
                    segsum_contrib(mrel[:, j * H:(j + 1) * H], t, rnd, zsh,
                                   selc, i0c)
                if msg_out is not None:
                    nc.sync.dma_start(msg_out[:, t0 * H:(t0 + k) * H],
                                      mrel[:])

        tc.strict_bb_all_engine_barrier()
        cur_zfull[0] = zfull2
        stage_mid(msg1, msg2, zfull1, zsh2, 2, None)
        tc.strict_bb_all_engine_barrier()
        stage_mid(msg2, None, zfull2, None, 3, None)

    nc.compile()
    return nc


# ---------------------------------------------------------------------------
# per-core input maps + output assembly
# ---------------------------------------------------------------------------

def np_dt(DT):
    import ml_dtypes
    return np.dtype(ml_dtypes.bfloat16) if DT == BF16 else np.float32


def make_in_maps(per_core, meta, W_i, W_h, W_o, b_o, DT=F32):
    T, NA = meta["T"], meta["NA"]
    d = np_dt(DT)
    weights = dict(
        wix0=W_i[:128], wix1=W_i[128:AF], wie=W_i[AF:],
        wh0=W_h[:128], wh1=W_h[128:256], wh2=W_h[256:],
        wox0=W_o[:128], wox1=W_o[128:AF],
        wom0=W_o[AF:AF + 128], wom1=W_o[AF + 128:AF + 256],
        wom2=W_o[AF + 256:],
    )
    weights = {k: np.ascontiguousarray(v.astype(d)) for k, v in weights.items()}
    iota = np.tile(np.arange(128, dtype=np.float32), (128, 1))
    bo = np.tile(b_o.astype(np.float32), (128, 1))
    maps = []
    X_pad2 = ceil_to(meta["X_pad"], 128)
    for pc in per_core:
        m = dict(weights)
        xsT = np.zeros((AF, X_pad2), dtype=d)
        xsT[:, : pc["x_sub"].shape[0]] = pc["x_sub"].T.astype(d)
        m["xsubT0"] = np.ascontiguousarray(xsT[:128])
        m["xsubT1"] = np.ascontiguousarray(xsT[128:])
        xoT = np.ascontiguousarray(pc["x_own"].T.astype(d))
        m["xownT0"] = np.ascontiguousarray(xoT[:128])
        m["xownT1"] = np.ascontiguousarray(xoT[128:])
        m["ea_in"] = np.ascontiguousarray(pc["ea"].T.astype(d))
        m["ctab_in"] = np.ascontiguousarray(pc["ctab"])
        m["zrow_in"] = np.ascontiguousarray(
            pc["zrow"].reshape(T, 128).T.astype(np.int32))
        m["xidx_in"] = np.ascontiguousarray(
            pc["xidx"].reshape(T, 128).T.astype(np.int32))
        m["gloc_in"] = np.ascontiguousarray(
            np.minimum(pc["gloc"], 1 << 20).reshape(NA, 128).T
            .astype(np.float32))
        m["iota_in"] = iota
        m["bo_in"] = bo
        maps.append(m)
    return maps


def assemble_mol(mol_parts, per_core, meta, num_graphs):
    out = np.zeros((num_graphs, H), dtype=np.float32)
    for c in range(C):
        g0s = per_core[c]["g0s"]
        for j in range(meta["NGRP"]):
            g0 = int(g0s[j])
            if g0 < 0:
                continue
            hi = min(g0 + 128, num_graphs)
            out[g0:hi] += mol_parts[c][j * 128: j * 128 + (hi - g0)]
    return out


# ---------------------------------------------------------------------------
# entry point
# ---------------------------------------------------------------------------

_prog_cache = {}


def _ensure_ntff_hook():
    """Register the axon NTFF profiling hook if the image's antenv lacks
    the axon_hooks module (profiling plumbing only; unused when
    trace=False)."""
    try:
        from antenv.axon_hooks import get_axon_ntff_profile_hook  # noqa
        return
    except ImportError:
        pass
    import types
    import antenv
    from trn_agent_boot.trn_boot import _ntff_profile_via_ctypes
    mod = types.ModuleType("antenv.axon_hooks")
    _h = [None]
    mod.set_axon_ntff_profile_hook = lambda h: _h.__setitem__(0, h)
    mod.get_axon_ntff_profile_hook = lambda: _h[0]
    sys.modules["antenv.axon_hooks"] = mod
    antenv.axon_hooks = mod
    try:
        hook = _ntff_profile_via_ctypes("/opt/axon/libaxon_pjrt.so")
        if hook is not None:
            mod.set_axon_ntff_profile_hook(hook)
    except Exception:
        pass
    # artifact upload needs a bucket; irrelevant for local profiling
    import concourse.bass_utils as _bu
    _bu.upload_artifacts = lambda tmpdir: tmpdir


def _run(inputs, DT=F32, trace=False, tmpdir=None):
    per_core, meta = preprocess(
        inputs["x"], inputs["edge_attr"], inputs["edge_index"],
        inputs["batch"], NUM_GRAPHS)
    key = (meta["T"], meta["NA"], meta["X_pad"], meta["NGRP"], str(DT),
           str(np.asarray(inputs["edge_index"])[:, 0]))
    ck = (meta["T"], meta["NA"], meta["X_pad"], meta["NGRP"], str(DT))
    if ck not in _prog_cache:
        _prog_cache[ck] = build_program(meta, DT=DT)
    nc = _prog_cache[ck]
    in_maps = make_in_maps(per_core, meta, inputs["W_i"], inputs["W_h"],
                           inputs["W_o"], inputs["b_o"], DT=DT)
    if trace:
        _ensure_ntff_hook()
    res = run_bass_kernel_spmd(nc, in_maps, list(range(C)), trace=trace,
                               tmpdir=tmpdir)
    mol_parts = [res.results[c]["molp"].astype(np.float32) for c in range(C)]
    out = assemble_mol(mol_parts, per_core, meta, NUM_GRAPHS)
    return out, res


def kernel(x, edge_attr, W_i, W_h, W_o, b_o, edge_index, batch):
    inputs = dict(x=np.asarray(x), edge_attr=np.asarray(edge_attr),
                  W_i=np.asarray(W_i), W_h=np.asarray(W_h),
                  W_o=np.asarray(W_o), b_o=np.asarray(b_o),
                  edge_index=np.asarray(edge_index),
                  batch=np.asarray(batch))
    out, _ = _run(inputs, DT=BF16)
    return out



# revision 12
# speedup vs baseline: 1.3066x; 1.3066x over previous
"""DMPNN encoder on 8 Trainium2 NeuronCores (Bass/Tile).

Strategy (data-parallel over graphs):
- Partition graphs into 8 contiguous chunks with ~equal atom counts
  (graph-aligned).  Each core owns the edges whose *target* atom lives in
  its chunk, sorted by local target -> segment-sum over targets is local.
- Per message-passing round, each core computes nei = segsum(msg) via
  selection-matrix matmuls, then Z = nei @ W_h on its own atoms, then the
  Z shards are AllGather'd across the 8 cores.  msg' = relu(msg + Z[src])
  only needs row gathers (indirect DMA) from the gathered Z table.
- Final round: atom_msg -> atom_h = relu([x||atom_msg] @ W_o + b_o) and
  sum-pool to graphs via selection matmuls; host sums partial group blocks.

All index manipulation is host-precomputed; the device does only dense
matmuls, elementwise ops, contiguous DMA and indirect row gathers.
"""

import os
import sys

for _p in ("/opt/trn_rl_repo", "/root/.axon_site/_ro/trn_rl_repo"):
    if os.path.isdir(_p) and _p not in sys.path:
        sys.path.insert(0, _p)

from contextlib import ExitStack

import numpy as np

import concourse.bass as bass
import concourse.tile as tile
from concourse import bacc, mybir
from concourse.bass_utils import run_bass_kernel_spmd
from concourse.masks import make_identity
from concourse.tile_rust import add_dep_helper

C = 8
H = 300
AF = 133
BF = 14
DEPTH = 3
NUM_GRAPHS = 4096

F32 = mybir.dt.float32
BF16 = mybir.dt.bfloat16
I32 = mybir.dt.int32

Relu = mybir.ActivationFunctionType.Relu
Copy = mybir.ActivationFunctionType.Copy
ADD = mybir.AluOpType.add
ISEQ = mybir.AluOpType.is_equal
BYPASS = mybir.AluOpType.bypass

IOA = bass.IndirectOffsetOnAxis


def ceil_to(x, m):
    return ((x + m - 1) // m) * m


# ---------------------------------------------------------------------------
# host-side preprocessing
# ---------------------------------------------------------------------------

def preprocess(x, edge_attr, edge_index, batch, num_graphs):
    N = x.shape[0]
    src = edge_index[0].astype(np.int64)
    tgt = edge_index[1].astype(np.int64)
    batch = batch.astype(np.int64)

    graph_start = np.searchsorted(batch, np.arange(num_graphs + 1))
    targets = (np.arange(1, C) * N) // C
    split_graphs = np.searchsorted(graph_start, targets)
    atom_splits = [0] + [int(graph_start[g]) for g in split_graphs] + [N]
    a0 = np.array(atom_splits[:-1])
    a1 = np.array(atom_splits[1:])
    n_real = a1 - a0
    N_pad = ceil_to(int(n_real.max()), 128) + 128
    NA = N_pad // 128

    owner = np.zeros(N, dtype=np.int64)
    loc = np.zeros(N, dtype=np.int64)
    for c in range(C):
        owner[a0[c]:a1[c]] = c
        loc[a0[c]:a1[c]] = np.arange(n_real[c])

    e_owner = owner[tgt]

    # per-core, per-ablock sorted edge lists (slot order within an ablock is
    # irrelevant for the selection matmuls)
    ab_eids = []  # [c][a] -> array of edge ids
    counts = np.zeros((C, NA), dtype=np.int64)
    for c in range(C):
        eids = np.nonzero(e_owner == c)[0]
        order = np.argsort(loc[tgt[eids]], kind="stable")
        eids = eids[order]
        ab = loc[tgt[eids]] // 128
        cuts = np.searchsorted(ab, np.arange(NA + 1))
        ab_eids.append([eids[cuts[a]:cuts[a + 1]] for a in range(NA)])
        counts[c] = cuts[1:] - cuts[:-1]

    for GRP in (64, 32, 16, 8, 4, 2, 1):
        ok = True
        for c in range(C):
            nb = int(n_real[c])
            for g0a in range(0, NA, GRP):
                alo, ahi = g0a * 128, min((g0a + GRP) * 128, nb)
                if alo >= nb:
                    continue
                if batch[a0[c] + ahi - 1] - batch[a0[c] + alo] >= 128:
                    ok = False
                    break
            if not ok:
                break
        if ok:
            break
    NGRP = (NA + GRP - 1) // GRP

    # greedy segmentation over ablocks: keep the union live-depth of nei
    # accumulators <= DMAX so only DMAX PSUM slots are needed
    DMAX = 3

    def seg_stats(s, e):
        P = np.zeros((C,), dtype=np.int64)
        first = {}
        last = {}
        for a in range(s, e):
            cnt = counts[:, a]
            act = cnt > 0
            if act.any():
                first[a] = int((P[act] // 128).min())
                last[a] = int(((P[act] + cnt[act] - 1) // 128).max())
            P += cnt
        ntile = int(-(-P.max() // 128)) if P.max() else 1
        depth = 0
        for t in range(ntile):
            depth = max(depth, sum(1 for a in first
                                   if first[a] <= t <= last[a]))
        return first, last, ntile, depth

    segments = []  # (a_start, a_end, first, last, ntiles)
    s = 0
    while s < NA:
        e = s + 1
        stats = seg_stats(s, e)
        while e < NA and (e % GRP) != 0:
            cand = seg_stats(s, e + 1)
            if cand[3] > DMAX:
                break
            e += 1
            stats = cand
        segments.append((s, e, stats[0], stats[1], stats[2]))
        s = e

    T = sum(sg[4] for sg in segments)
    Epad = T * 128

    contribs = [[] for _ in range(T)]
    base = 0
    for (s, e, first, last, ntiles) in segments:
        for a in range(s, e):
            if a in first:
                ts = range(base + first[a], base + last[a] + 1)
            else:
                ts = [base]  # no edges anywhere: zero matmul, still drains
            ts = list(ts)
            for i, t in enumerate(ts):
                contribs[t].append((a, i == 0, i == len(ts) - 1))
        base += ntiles

    # contrib enumeration order + chunk boundaries for batched sel builds
    contrib_idx = {}
    ci = 0
    for t in range(T):
        for (a, f, l) in contribs[t]:
            contrib_idx[(t, a)] = ci
            ci += 1
    NCONTRIB = ci

    drain_seq = [a for t in range(T) for (a, f, l) in contribs[t] if l]
    pool_first = {}
    pool_last = {}
    seen = {}
    for a in drain_seq:
        j = a // GRP
        if j not in seen:
            pool_first[a] = True
        seen[j] = a
    for j, a in seen.items():
        pool_last[a] = True

    # collective chunking: chunk-major Z-table layout so each chunk's
    # AllGather output is a contiguous region.  Sizes decrease so the last
    # chunk's exchange latency (the only un-overlappable part) is tiny.
    gu = (NA + GRP - 1) // GRP  # group units
    unit_counts = []
    rem = gu
    for frac in (3, 3, 4, 6, 10, 16):
        take = max(1, round(rem_part) if (rem_part := gu / frac) and False else
                   max(1, gu // frac))
        take = min(take, rem)
        if take:
            unit_counts.append(take)
            rem -= take
        if rem == 0:
            break
    while rem > 0:
        unit_counts.append(1)
        rem -= 1
    cc_chunks = []
    a = 0
    for u in unit_counts:
        e = min(a + u * GRP, NA)
        cc_chunks.append((a, e))
        a = e
        if a >= NA:
            break
    n_cc = len(cc_chunks)
    cc_start = np.array([k0 * 128 for (k0, k1) in cc_chunks])
    cc_rows = np.array([(k1 - k0) * 128 for (k0, k1) in cc_chunks])
    cc_off = np.concatenate([[0], np.cumsum(C * cc_rows)])[:-1]

    # lay out slots: per core, segments concatenated, padded to common size
    per_core = []
    for c in range(C):
        slot_eid = np.full(Epad, -1, dtype=np.int64)
        base = 0
        for (s, e, first, last, ntiles) in segments:
            seg = np.concatenate([ab_eids[c][a] for a in range(s, e)]) \
                if e > s else np.zeros(0, np.int64)
            slot_eid[base:base + len(seg)] = seg
            base += ntiles * 128
        real = slot_eid >= 0
        eids = slot_eid[real]
        ne = int(real.sum())
        tloc = np.full(Epad, -1, dtype=np.int64)
        tloc[real] = loc[tgt[eids]]
        src_g = np.zeros(Epad, dtype=np.int64)
        src_g[real] = src[eids]
        ea = np.zeros((Epad, BF), dtype=np.float32)
        ea[real] = edge_attr[eids]
        zrow = np.zeros(Epad, dtype=np.int64)
        ol = owner[src_g[real]]
        ll = loc[src_g[real]]
        ck = np.searchsorted(cc_start, ll, side="right") - 1
        zrow[real] = cc_off[ck] + ol * cc_rows[ck] + (ll - cc_start[ck])
        uniq, inv = np.unique(src_g[real], return_inverse=True)
        xidx = np.zeros(Epad, dtype=np.int64)
        xidx[real] = inv
        xcat = np.zeros((Epad, AF + BF), dtype=np.float32)
        xcat[:, :AF] = x[src_g]
        xcat[:, AF:] = ea
        ctab = np.zeros(NCONTRIB, dtype=object)
        ctabm = np.zeros((128, NCONTRIB), dtype=np.float32)
        for t in range(T):
            for (a, f, l) in contribs[t]:
                i = contrib_idx[(t, a)]
                ctabm[:, i] = (tloc[t * 128:(t + 1) * 128] - 128 * a)
        per_core.append(dict(ne=ne, real=real, tloc=tloc, ea=ea, zrow=zrow,
                             xidx=xidx, uniq=uniq, n_uniq=len(uniq),
                             ctab=ctabm, xcat=xcat))

    X_pad = max(pc["n_uniq"] for pc in per_core) + 1
    for pc in per_core:
        pc["xidx"][~pc["real"]] = X_pad - 1
        xs = np.zeros((X_pad, AF), dtype=np.float32)
        xs[: pc["n_uniq"]] = x[pc["uniq"]]
        pc["x_sub"] = xs

    for c in range(C):
        gloc = np.full(N_pad, 1 << 20, dtype=np.int64)
        g0s = np.full(NGRP, -1, dtype=np.int64)
        nb = int(n_real[c])
        for j in range(NGRP):
            alo = j * GRP * 128
            ahi = min(alo + GRP * 128, nb)
            if alo >= nb:
                continue
            g0 = batch[a0[c] + alo]
            g0s[j] = g0
            gloc[alo:ahi] = batch[a0[c] + alo: a0[c] + ahi] - g0
        per_core[c]["gloc"] = gloc
        per_core[c]["g0s"] = g0s
        xo = np.zeros((N_pad, AF), dtype=np.float32)
        xo[:nb] = x[a0[c]: a1[c]]
        per_core[c]["x_own"] = xo

    meta = dict(N_pad=N_pad, NA=NA, Epad=Epad, T=T, X_pad=X_pad, GRP=GRP,
                NGRP=NGRP, a0=a0, a1=a1, n_real=n_real, contribs=contribs,
                pool_first=pool_first, pool_last=pool_last,
                contrib_idx=contrib_idx, NCONTRIB=NCONTRIB,
                cc_chunks=cc_chunks, cc_off=cc_off, cc_rows=cc_rows)
    return per_core, meta


# ---------------------------------------------------------------------------
# the Bass program (identical for all 8 cores; data differs per core)
# ---------------------------------------------------------------------------

def build_program(meta, DT=F32, CH=4):
    T, NA, N_pad = meta["T"], meta["NA"], meta["N_pad"]
    Epad, X_pad = meta["Epad"], meta["X_pad"]
    GRP, NGRP = meta["GRP"], meta["NGRP"]
    contribs = meta["contribs"]

    nc = bacc.Bacc("TRN2", target_bir_lowering=False, debug=False,
                   num_devices=C)
    DTZ = mybir.dt.float8e4  # Z-exchange dtype (halves collective traffic)

    xcatT0 = nc.dram_tensor("xcatT0", [128, Epad], DT, kind="ExternalInput")
    xcatT1 = nc.dram_tensor("xcatT1", [AF + BF - 128, Epad], DT,
                            kind="ExternalInput")
    xownT0 = nc.dram_tensor("xownT0", [128, N_pad], DT, kind="ExternalInput")
    xownT1 = nc.dram_tensor("xownT1", [AF - 128, N_pad], DT,
                            kind="ExternalInput")
    ctab_in = nc.dram_tensor("ctab_in", [128, meta["NCONTRIB"]], F32,
                             kind="ExternalInput")
    zrow_in = nc.dram_tensor("zrow_in", [128, T], I32, kind="ExternalInput")
    gloc_in = nc.dram_tensor("gloc_in", [128, NA], F32, kind="ExternalInput")
    iota_in = nc.dram_tensor("iota_in", [128, 128], F32, kind="ExternalInput")
    bo_in = nc.dram_tensor("bo_in", [128, H], F32, kind="ExternalInput")

    wname_shapes = dict(
        wix0=[128, H], wcat1=[AF + BF - 128, H],
        wh0=[128, H], wh1=[128, H], wh2=[H - 256, H],
        wox0=[128, H], wox1=[AF - 128, H],
        wom0=[128, H], wom1=[128, H], wom2=[H - 256, H],
    )
    w_in = {k: nc.dram_tensor(k, s, DT, kind="ExternalInput")
            for k, s in wname_shapes.items()}

    molp = nc.dram_tensor("molp", [NGRP * 128, H], F32, kind="ExternalOutput")

    zfull1 = nc.dram_tensor("zfull1", [C * N_pad, H], DTZ, addr_space="Shared")
    zfull2 = nc.dram_tensor("zfull2", [C * N_pad, H], DTZ, addr_space="Shared")
    cc_chunks = meta["cc_chunks"]

    HC = [(0, 128), (128, 256), (256, H)]  # hidden-dim K chunks
    XC = [(0, 128), (128, AF)]             # atom-feature K chunks

    with tile.TileContext(nc) as tc, ExitStack() as ctx:
        const = ctx.enter_context(tc.tile_pool(name="const", bufs=1))
        sb = ctx.enter_context(tc.tile_pool(name="sb", bufs=4))
        ps = ctx.enter_context(tc.tile_pool(name="ps", bufs=2, space="PSUM"))
        dram = ctx.enter_context(tc.tile_pool(name="dram", bufs=1,
                                              space="DRAM"))

        # ---- residents ----
        def cload(name, src, shape, dt):
            tl = const.tile(shape, dt, tag=name)
            nc.sync.dma_start(tl[:], src[:])
            return tl

        ctab_s = cload("ctab", ctab_in, [128, meta["NCONTRIB"]], F32)
        zrow_s = cload("zrow", zrow_in, [128, T], I32)
        gloc_s = cload("gloc", gloc_in, [128, NA], F32)
        iota_s = cload("iota", iota_in, [128, 128], F32)
        bo_s = cload("bo", bo_in, [128, H], F32)
        w_s = {k: cload(k, w_in[k], wname_shapes[k], DT) for k in w_in}
        identF = const.tile([128, 128], F32, tag="identF")
        make_identity(nc, identF[:])


        msg1 = dram.tile([128, T * H], DT, tag="msg1")
        msg2 = dram.tile([128, T * H], DT, tag="msg2")
        zsh1 = dram.tile([N_pad, H], DTZ, tag="zsh1")
        zsh2 = dram.tile([N_pad, H], DTZ, tag="zsh2")

        psum_nei = {}
        psum_pool = {}

        def transpose_chunks(src_ap, chunks, ident):
            """PE-transpose column chunks of src_ap -> list of SBUF DT tiles."""
            out = []
            for (c0, c1) in chunks:
                w = c1 - c0
                tp = ps.tile([128, 128], F32, tag="pB", bufs=2)
                nc.tensor.transpose(tp[:w, :], src_ap[:, c0:c1], ident[:])
                trs = sb.tile([128, 128], DT, tag="trs")
                nc.vector.tensor_copy(trs[:w, :], tp[:w, :])
                out.append(trs)
            return out

        cc_state = {}

        cur_zfull = [None]

        def drain_z(a, zsh):
            """psum_nei[a] -> Z = nei @ W_h -> zsh rows of ablock a."""
            nei_sb = sb.tile([128, H], F32, tag="neisb")
            nc.vector.tensor_copy(nei_sb[:], psum_nei.pop(a)[:])
            trs = transpose_chunks(nei_sb, HC, identF)
            zps = ps.tile([128, H], F32, tag="pD", bufs=1)
            for ci, ((c0, c1), wt) in enumerate(
                    zip(HC, (w_s["wh0"], w_s["wh1"], w_s["wh2"]))):
                w = c1 - c0
                nc.tensor.matmul(zps[:], lhsT=trs[ci][:w, :], rhs=wt[:],
                                 start=(ci == 0), stop=(ci == len(HC) - 1))
            zsb = sb.tile([128, H], DTZ, tag="zsb")
            nc.scalar.copy(zsb[:], zps[:])
            nc.sync.dma_start(zsh[a * 128:(a + 1) * 128, :], zsb[:])
            fire_cc(a, zsh, cur_zfull[0])

        def drain_final(a):
            """psum_nei[a] = atom_msg -> atom_h -> pool into group psum."""
            am_sb = sb.tile([128, H], F32, tag="neisb")
            nc.vector.tensor_copy(am_sb[:], psum_nei.pop(a)[:])
            hps = ps.tile([128, H], F32, tag="pA", bufs=2)
            at = transpose_chunks(am_sb, HC, identF)
            asl = slice(a * 128, (a + 1) * 128)
            xo0 = sb.tile([128, 128], DT, tag="xo0")
            nc.sync.dma_start(xo0[:], xownT0[:, asl])
            xo1 = sb.tile([AF - 128, 128], DT, tag="xo1")
            nc.sync.dma_start(xo1[:], xownT1[:, asl])
            parts = [(XC[0], xo0[:], w_s["wox0"]),
                     (XC[1], xo1[:], w_s["wox1"])] + \
                [((c0, c1), at[ci][:c1 - c0, :], w_s[k])
                 for ci, ((c0, c1), k) in enumerate(
                     zip(HC, ("wom0", "wom1", "wom2")))]
            for ci, ((c0, c1), lh, wt) in enumerate(parts):
                nc.tensor.matmul(hps[:], lhsT=lh, rhs=wt[:],
                                 start=(ci == 0), stop=(ci == len(parts) - 1))
            hsum = sb.tile([128, H], DT, tag="hsum")
            nc.vector.tensor_tensor(hsum[:], hps[:], bo_s[:], op=ADD)
            hrelu = sb.tile([128, H], DT, tag="hrelu")
            nc.scalar.activation(hrelu[:], hsum[:], Relu)
            selp = sb.tile([128, 128], DT, tag="selp")
            nc.vector.tensor_tensor(
                selp[:], gloc_s[:, a:a + 1].to_broadcast([128, 128]),
                iota_s[:], op=ISEQ)
            j = a // GRP
            first = meta["pool_first"].get(a, False)
            last = meta["pool_last"].get(a, False)
            if first:
                psum_pool[j] = ps.tile([128, H], F32, tag="pD", name=f"pool{j}", bufs=1)
            nc.tensor.matmul(psum_pool[j][:], lhsT=selp[:], rhs=hrelu[:],
                             start=first, stop=last)
            if last:
                mol_sb = sb.tile([128, H], F32, tag="molsb")
                nc.vector.tensor_copy(mol_sb[:], psum_pool.pop(j)[:])
                nc.sync.dma_start(molp[j * 128:(j + 1) * 128, :], mol_sb[:])

        contrib_idx = meta["contrib_idx"]
        max_ncc = max(sum(len(contribs[t]) for t in range(t0, min(t0 + CH, T)))
                      for t0 in range(0, T, CH))

        def build_sels(t0, k):
            """One DVE op building all sel matrices for tiles [t0, t0+k)."""
            idxs = [contrib_idx[(t, a)] for t in range(t0, t0 + k)
                    for (a, f, l) in contribs[t]]
            if not idxs:
                return None, 0
            i0, ncc = idxs[0], len(idxs)
            assert idxs == list(range(i0, i0 + ncc))
            selc = sb.tile([128, ncc * 128], DT, tag="sel",
                           padded_shape=[128, max_ncc * 128], name="selc")
            base = ctab_s[:, i0:i0 + ncc]
            in0 = bass.AP(tensor=base.tensor, offset=base.offset,
                          ap=list(base.ap) + [[0, 128]])
            it = iota_s[:]
            in1 = bass.AP(tensor=it.tensor, offset=it.offset,
                          ap=[it.ap[0], [0, ncc], it.ap[1]])
            out3 = selc.rearrange("p (a b) -> p a b", b=128)
            nc.vector.tensor_tensor(out3, in0, in1, op=ISEQ)
            return selc, i0

        def segsum_contrib(msg_ap, t, rnd, zsh, selc, i0):
            for (a, first, last) in contribs[t]:
                q = contrib_idx[(t, a)] - i0
                sel_ap = selc[:, q * 128:(q + 1) * 128]
                if first:
                    psum_nei[a] = ps.tile([128, H], F32, tag="pC", name=f"nei{a}", bufs=3)
                nc.tensor.matmul(psum_nei[a][:], lhsT=sel_ap, rhs=msg_ap,
                                 start=first, stop=last)
                if last:
                    if rnd < DEPTH:
                        drain_z(a, zsh)
                    else:
                        drain_final(a)

        # ---- stage A: initial messages + round-1 segsum ----
        def fire_cc(a, zsh, zfull):
            """After ablock a's drain DMA: if it completes a cc chunk, fire
            that chunk's AllGather."""
            st = cc_state.setdefault(id(zsh), dict(done=set()))
            st["done"].add(a)
            for (k0, k1) in cc_chunks:
                if all(x in st["done"] for x in range(k0, k1)) \
                        and (k0, k1) not in st.get("fired", set()):
                    st.setdefault("fired", set()).add((k0, k1))
                    ci = cc_chunks.index((k0, k1))
                    off = int(meta["cc_off"][ci])
                    nrows = int(meta["cc_rows"][ci])
                    cc = nc.gpsimd.collective_compute(
                        "AllGather", BYPASS,
                        replica_groups=[list(range(C))],
                        ins=[zsh[k0 * 128:k1 * 128, :]],
                        outs=[zfull[off:off + C * nrows, :]])
                    st["last_cc"] = cc

        # ---- stage A: msg1 = relu([x[src]||ea] @ W_i) + round-1 segsum ----
        # host supplies the per-slot concat table transposed; two matmuls
        # accumulate in PSUM and relu reads PSUM directly.
        cur_zfull[0] = zfull1
        for t0 in range(0, T, CH):
            k = min(CH, T - t0)
            selc, i0c = build_sels(t0, k)
            csl = slice(t0 * 128, (t0 + k) * 128)
            xc0 = sb.tile([128, k * 128], DT, tag="xc0", bufs=3)
            nc.sync.dma_start(xc0[:], xcatT0[:, csl])
            xc1 = sb.tile([AF + BF - 128, k * 128], DT, tag="xc1", bufs=3)
            nc.sync.dma_start(xc1[:], xcatT1[:, csl])
            msg_sb = sb.tile([128, k * H], DT, tag="msg", bufs=3)
            for j in range(k):
                t = t0 + j
                jsl = slice(j * 128, (j + 1) * 128)
                mps = ps.tile([128, H], F32, tag="pA", bufs=2)
                nc.tensor.matmul(mps[:], lhsT=xc0[:, jsl],
                                 rhs=w_s["wix0"][:], start=True, stop=False)
                nc.tensor.matmul(mps[:], lhsT=xc1[:, jsl],
                                 rhs=w_s["wcat1"][:], start=False, stop=True)
                m_ap = msg_sb[:, j * H:(j + 1) * H]
                nc.scalar.activation(m_ap, mps[:], Relu)
                segsum_contrib(m_ap, t, 1, zsh1, selc, i0c)
            nc.sync.dma_start(msg1[:, t0 * H:(t0 + k) * H], msg_sb[:])

        # ---- stages B (round 2) and C (round 3 + readout) ----
        def stage_mid(msg_in, msg_out, zfull, zsh, rnd, cc_prev):
            zflat = zfull[:]
            for t0 in range(0, T, CH):
                k = min(CH, T - t0)
                selc, i0c = build_sels(t0, k)
                ld = sb.tile([128, k * H], DT, tag="ld", bufs=3)
                nc.sync.dma_start(ld[:], msg_in[:, t0 * H:(t0 + k) * H])
                mrel = sb.tile([128, k * H], DT, tag="msg", bufs=3)
                for j in range(k):
                    t = t0 + j
                    zg = sb.tile([128, H], DTZ, tag="zg", bufs=6)
                    nc.gpsimd.indirect_dma_start(
                        out=zg[:], out_offset=None, in_=zflat,
                        in_offset=IOA(ap=zrow_s[:, t:t + 1], axis=0))
                    msum = sb.tile([128, H], DT, tag="msum")
                    nc.vector.tensor_tensor(msum[:], ld[:, j * H:(j + 1) * H],
                                            zg[:], op=ADD)
                    nc.scalar.activation(mrel[:, j * H:(j + 1) * H], msum[:],
                                         Relu)
                    segsum_contrib(mrel[:, j * H:(j + 1) * H], t, rnd, zsh,
                                   selc, i0c)
                if msg_out is not None:
                    nc.sync.dma_start(msg_out[:, t0 * H:(t0 + k) * H],
                                      mrel[:])

        tc.strict_bb_all_engine_barrier()
        cur_zfull[0] = zfull2
        stage_mid(msg1, msg2, zfull1, zsh2, 2, None)
        tc.strict_bb_all_engine_barrier()
        stage_mid(msg2, None, zfull2, None, 3, None)

    nc.compile()
    return nc


# ---------------------------------------------------------------------------
# per-core input maps + output assembly
# ---------------------------------------------------------------------------

def np_dt(DT):
    import ml_dtypes
    return np.dtype(ml_dtypes.bfloat16) if DT == BF16 else np.float32


def make_in_maps(per_core, meta, W_i, W_h, W_o, b_o, DT=F32):
    T, NA = meta["T"], meta["NA"]
    d = np_dt(DT)
    weights = dict(
        wix0=W_i[:128], wcat1=W_i[128:],
        wh0=W_h[:128], wh1=W_h[128:256], wh2=W_h[256:],
        wox0=W_o[:128], wox1=W_o[128:AF],
        wom0=W_o[AF:AF + 128], wom1=W_o[AF + 128:AF + 256],
        wom2=W_o[AF + 256:],
    )
    weights = {k: np.ascontiguousarray(v.astype(d)) for k, v in weights.items()}
    iota = np.tile(np.arange(128, dtype=np.float32), (128, 1))
    bo = np.tile(b_o.astype(np.float32), (128, 1))
    maps = []
    for pc in per_core:
        m = dict(weights)
        xcT = np.ascontiguousarray(pc["xcat"].T.astype(d))
        m["xcatT0"] = np.ascontiguousarray(xcT[:128])
        m["xcatT1"] = np.ascontiguousarray(xcT[128:])
        xoT = np.ascontiguousarray(pc["x_own"].T.astype(d))
        m["xownT0"] = np.ascontiguousarray(xoT[:128])
        m["xownT1"] = np.ascontiguousarray(xoT[128:])
        m["ctab_in"] = np.ascontiguousarray(pc["ctab"])
        m["zrow_in"] = np.ascontiguousarray(
            pc["zrow"].reshape(T, 128).T.astype(np.int32))
        m["gloc_in"] = np.ascontiguousarray(
            np.minimum(pc["gloc"], 1 << 20).reshape(NA, 128).T
            .astype(np.float32))
        m["iota_in"] = iota
        m["bo_in"] = bo
        maps.append(m)
    return maps


def assemble_mol(mol_parts, per_core, meta, num_graphs):
    out = np.zeros((num_graphs, H), dtype=np.float32)
    for c in range(C):
        g0s = per_core[c]["g0s"]
        for j in range(meta["NGRP"]):
            g0 = int(g0s[j])
            if g0 < 0:
                continue
            hi = min(g0 + 128, num_graphs)
            out[g0:hi] += mol_parts[c][j * 128: j * 128 + (hi - g0)]
    return out


# ---------------------------------------------------------------------------
# entry point
# ---------------------------------------------------------------------------

_prog_cache = {}


def _ensure_ntff_hook():
    """Register the axon NTFF profiling hook if the image's antenv lacks
    the axon_hooks module (profiling plumbing only; unused when
    trace=False)."""
    try:
        from antenv.axon_hooks import get_axon_ntff_profile_hook  # noqa
        return
    except ImportError:
        pass
    import types
    import antenv
    from trn_agent_boot.trn_boot import _ntff_profile_via_ctypes
    mod = types.ModuleType("antenv.axon_hooks")
    _h = [None]
    mod.set_axon_ntff_profile_hook = lambda h: _h.__setitem__(0, h)
    mod.get_axon_ntff_profile_hook = lambda: _h[0]
    sys.modules["antenv.axon_hooks"] = mod
    antenv.axon_hooks = mod
    try:
        hook = _ntff_profile_via_ctypes("/opt/axon/libaxon_pjrt.so")
        if hook is not None:
            mod.set_axon_ntff_profile_hook(hook)
    except Exception:
        pass
    # artifact upload needs a bucket; irrelevant for local profiling
    import concourse.bass_utils as _bu
    _bu.upload_artifacts = lambda tmpdir: tmpdir


def _run(inputs, DT=F32, trace=False, tmpdir=None):
    per_core, meta = preprocess(
        inputs["x"], inputs["edge_attr"], inputs["edge_index"],
        inputs["batch"], NUM_GRAPHS)
    key = (meta["T"], meta["NA"], meta["X_pad"], meta["NGRP"], str(DT),
           str(np.asarray(inputs["edge_index"])[:, 0]))
    ck = (meta["T"], meta["NA"], meta["X_pad"], meta["NGRP"], str(DT))
    if ck not in _prog_cache:
        _prog_cache[ck] = build_program(meta, DT=DT)
    nc = _prog_cache[ck]
    in_maps = make_in_maps(per_core, meta, inputs["W_i"], inputs["W_h"],
                           inputs["W_o"], inputs["b_o"], DT=DT)
    if trace:
        _ensure_ntff_hook()
    res = run_bass_kernel_spmd(nc, in_maps, list(range(C)), trace=trace,
                               tmpdir=tmpdir)
    mol_parts = [res.results[c]["molp"].astype(np.float32) for c in range(C)]
    out = assemble_mol(mol_parts, per_core, meta, NUM_GRAPHS)
    return out, res


def kernel(x, edge_attr, W_i, W_h, W_o, b_o, edge_index, batch):
    inputs = dict(x=np.asarray(x), edge_attr=np.asarray(edge_attr),
                  W_i=np.asarray(W_i), W_h=np.asarray(W_h),
                  W_o=np.asarray(W_o), b_o=np.asarray(b_o),
                  edge_index=np.asarray(edge_index),
                  batch=np.asarray(batch))
    out, _ = _run(inputs, DT=BF16)
    return out



# revision 23
# speedup vs baseline: 1.3140x; 1.0057x over previous
"""DMPNN encoder on 8 Trainium2 NeuronCores (Bass/Tile).

Strategy (data-parallel over graphs):
- Partition graphs into 8 contiguous chunks with ~equal atom counts
  (graph-aligned).  Each core owns the edges whose *target* atom lives in
  its chunk, sorted by local target -> segment-sum over targets is local.
- Per message-passing round, each core computes nei = segsum(msg) via
  selection-matrix matmuls, then Z = nei @ W_h on its own atoms, then the
  Z shards are AllGather'd across the 8 cores.  msg' = relu(msg + Z[src])
  only needs row gathers (indirect DMA) from the gathered Z table.
- Final round: atom_msg -> atom_h = relu([x||atom_msg] @ W_o + b_o) and
  sum-pool to graphs via selection matmuls; host sums partial group blocks.

All index manipulation is host-precomputed; the device does only dense
matmuls, elementwise ops, contiguous DMA and indirect row gathers.
"""

import os
import sys

for _p in ("/opt/trn_rl_repo", "/root/.axon_site/_ro/trn_rl_repo"):
    if os.path.isdir(_p) and _p not in sys.path:
        sys.path.insert(0, _p)

from contextlib import ExitStack

import numpy as np

import concourse.bass as bass
import concourse.tile as tile
from concourse import bacc, mybir
from concourse.bass_utils import run_bass_kernel_spmd
from concourse.masks import make_identity
from concourse.tile_rust import add_dep_helper

C = 8
H = 300
AF = 133
BF = 14
DEPTH = 3
NUM_GRAPHS = 4096

F32 = mybir.dt.float32
BF16 = mybir.dt.bfloat16
I32 = mybir.dt.int32

Relu = mybir.ActivationFunctionType.Relu
Copy = mybir.ActivationFunctionType.Copy
ADD = mybir.AluOpType.add
ISEQ = mybir.AluOpType.is_equal
BYPASS = mybir.AluOpType.bypass

IOA = bass.IndirectOffsetOnAxis


def ceil_to(x, m):
    return ((x + m - 1) // m) * m


# ---------------------------------------------------------------------------
# host-side preprocessing
# ---------------------------------------------------------------------------

def preprocess(x, edge_attr, edge_index, batch, num_graphs):
    N = x.shape[0]
    src = edge_index[0].astype(np.int64)
    tgt = edge_index[1].astype(np.int64)
    batch = batch.astype(np.int64)

    graph_start = np.searchsorted(batch, np.arange(num_graphs + 1))
    targets = (np.arange(1, C) * N) // C
    split_graphs = np.searchsorted(graph_start, targets)
    atom_splits = [0] + [int(graph_start[g]) for g in split_graphs] + [N]
    a0 = np.array(atom_splits[:-1])
    a1 = np.array(atom_splits[1:])
    n_real = a1 - a0
    N_pad = ceil_to(int(n_real.max()), 128) + 128
    NA = N_pad // 128

    owner = np.zeros(N, dtype=np.int64)
    loc = np.zeros(N, dtype=np.int64)
    for c in range(C):
        owner[a0[c]:a1[c]] = c
        loc[a0[c]:a1[c]] = np.arange(n_real[c])

    e_owner = owner[tgt]

    # per-core, per-ablock sorted edge lists (slot order within an ablock is
    # irrelevant for the selection matmuls)
    ab_eids = []  # [c][a] -> array of edge ids
    counts = np.zeros((C, NA), dtype=np.int64)
    for c in range(C):
        eids = np.nonzero(e_owner == c)[0]
        order = np.argsort(loc[tgt[eids]], kind="stable")
        eids = eids[order]
        ab = loc[tgt[eids]] // 128
        cuts = np.searchsorted(ab, np.arange(NA + 1))
        ab_eids.append([eids[cuts[a]:cuts[a + 1]] for a in range(NA)])
        counts[c] = cuts[1:] - cuts[:-1]

    for GRP in (64, 32, 16, 8, 4, 2, 1):
        ok = True
        for c in range(C):
            nb = int(n_real[c])
            for g0a in range(0, NA, GRP):
                alo, ahi = g0a * 128, min((g0a + GRP) * 128, nb)
                if alo >= nb:
                    continue
                if batch[a0[c] + ahi - 1] - batch[a0[c] + alo] >= 128:
                    ok = False
                    break
            if not ok:
                break
        if ok:
            break
    NGRP = (NA + GRP - 1) // GRP

    # greedy segmentation over ablocks: keep the union live-depth of nei
    # accumulators <= DMAX so only DMAX PSUM slots are needed
    DMAX = 3

    def seg_stats(s, e):
        P = np.zeros((C,), dtype=np.int64)
        first = {}
        last = {}
        for a in range(s, e):
            cnt = counts[:, a]
            act = cnt > 0
            if act.any():
                first[a] = int((P[act] // 128).min())
                last[a] = int(((P[act] + cnt[act] - 1) // 128).max())
            P += cnt
        ntile = int(-(-P.max() // 128)) if P.max() else 1
        depth = 0
        for t in range(ntile):
            depth = max(depth, sum(1 for a in first
                                   if first[a] <= t <= last[a]))
        return first, last, ntile, depth

    segments = []  # (a_start, a_end, first, last, ntiles)
    s = 0
    while s < NA:
        e = s + 1
        stats = seg_stats(s, e)
        while e < NA and (e % GRP) != 0:
            cand = seg_stats(s, e + 1)
            if cand[3] > DMAX:
                break
            e += 1
            stats = cand
        segments.append((s, e, stats[0], stats[1], stats[2]))
        s = e

    T = sum(sg[4] for sg in segments)
    Epad = T * 128

    contribs = [[] for _ in range(T)]
    base = 0
    for (s, e, first, last, ntiles) in segments:
        for a in range(s, e):
            if a in first:
                ts = range(base + first[a], base + last[a] + 1)
            else:
                ts = [base]  # no edges anywhere: zero matmul, still drains
            ts = list(ts)
            for i, t in enumerate(ts):
                contribs[t].append((a, i == 0, i == len(ts) - 1))
        base += ntiles

    # contrib enumeration order + chunk boundaries for batched sel builds
    contrib_idx = {}
    ci = 0
    for t in range(T):
        for (a, f, l) in contribs[t]:
            contrib_idx[(t, a)] = ci
            ci += 1
    NCONTRIB = ci

    drain_seq = [a for t in range(T) for (a, f, l) in contribs[t] if l]
    pool_first = {}
    pool_last = {}
    seen = {}
    for a in drain_seq:
        j = a // GRP
        if j not in seen:
            pool_first[a] = True
        seen[j] = a
    for j, a in seen.items():
        pool_last[a] = True

    # collective chunking: chunk-major Z-table layout so each chunk's
    # AllGather output is a contiguous region.  Sizes decrease so the last
    # chunk's exchange latency (the only un-overlappable part) is tiny.
    gu = (NA + GRP - 1) // GRP  # group units
    unit_counts = []
    rem = gu
    for frac in (6, 6, 7, 8, 9, 10, 12, 14, 16, 16):
        take = min(max(1, gu // frac), rem)
        if take:
            unit_counts.append(take)
            rem -= take
        if rem == 0:
            break
    while rem > 0:
        unit_counts.append(1)
        rem -= 1
    cc_chunks = []
    a = 0
    for u in unit_counts:
        e = min(a + u * GRP, NA)
        cc_chunks.append((a, e))
        a = e
        if a >= NA:
            break
    n_cc = len(cc_chunks)
    cc_start = np.array([k0 * 128 for (k0, k1) in cc_chunks])
    cc_rows = np.array([(k1 - k0) * 128 for (k0, k1) in cc_chunks])
    cc_off = np.concatenate([[0], np.cumsum(C * cc_rows)])[:-1]

    # lay out slots: per core, segments concatenated, padded to common size
    per_core = []
    for c in range(C):
        slot_eid = np.full(Epad, -1, dtype=np.int64)
        base = 0
        for (s, e, first, last, ntiles) in segments:
            seg = np.concatenate([ab_eids[c][a] for a in range(s, e)]) \
                if e > s else np.zeros(0, np.int64)
            slot_eid[base:base + len(seg)] = seg
            base += ntiles * 128
        real = slot_eid >= 0
        eids = slot_eid[real]
        ne = int(real.sum())
        tloc = np.full(Epad, -1, dtype=np.int64)
        tloc[real] = loc[tgt[eids]]
        src_g = np.zeros(Epad, dtype=np.int64)
        src_g[real] = src[eids]
        ea = np.zeros((Epad, BF), dtype=np.float32)
        ea[real] = edge_attr[eids]
        zrow = np.zeros(Epad, dtype=np.int64)
        ol = owner[src_g[real]]
        ll = loc[src_g[real]]
        ck = np.searchsorted(cc_start, ll, side="right") - 1
        zrow[real] = cc_off[ck] + ol * cc_rows[ck] + (ll - cc_start[ck])
        uniq, inv = np.unique(src_g[real], return_inverse=True)
        xidx = np.zeros(Epad, dtype=np.int64)
        xidx[real] = inv
        xcat = np.zeros((Epad, AF + BF), dtype=np.float32)
        xcat[:, :AF] = x[src_g]
        xcat[:, AF:] = ea
        ctabm = np.zeros((128, NCONTRIB), dtype=np.float32)
        for t in range(T):
            for (a, f, l) in contribs[t]:
                i = contrib_idx[(t, a)]
                ctabm[:, i] = (tloc[t * 128:(t + 1) * 128] - 128 * a)
        selm = (ctabm[:, :, None] == np.arange(128, dtype=np.float32)
                ).astype(np.float32).reshape(128, NCONTRIB * 128)
        per_core.append(dict(ne=ne, real=real, tloc=tloc, ea=ea, zrow=zrow,
                             xidx=xidx, uniq=uniq, n_uniq=len(uniq),
                             selm=selm, xcat=xcat))

    X_pad = max(pc["n_uniq"] for pc in per_core) + 1
    for pc in per_core:
        pc["xidx"][~pc["real"]] = X_pad - 1
        xs = np.zeros((X_pad, AF), dtype=np.float32)
        xs[: pc["n_uniq"]] = x[pc["uniq"]]
        pc["x_sub"] = xs

    for c in range(C):
        gloc = np.full(N_pad, 1 << 20, dtype=np.int64)
        g0s = np.full(NGRP, -1, dtype=np.int64)
        nb = int(n_real[c])
        for j in range(NGRP):
            alo = j * GRP * 128
            ahi = min(alo + GRP * 128, nb)
            if alo >= nb:
                continue
            g0 = batch[a0[c] + alo]
            g0s[j] = g0
            gloc[alo:ahi] = batch[a0[c] + alo: a0[c] + ahi] - g0
        per_core[c]["g0s"] = g0s
        glocm = np.minimum(gloc, 1 << 20).reshape(NA, 128).T.astype(np.float32)
        per_core[c]["selp"] = (
            glocm[:, :, None] == np.arange(128, dtype=np.float32)
        ).astype(np.float32).reshape(128, NA * 128)
        xo = np.zeros((N_pad, AF), dtype=np.float32)
        xo[:nb] = x[a0[c]: a1[c]]
        per_core[c]["x_own"] = xo

    meta = dict(N_pad=N_pad, NA=NA, Epad=Epad, T=T, X_pad=X_pad, GRP=GRP,
                NGRP=NGRP, a0=a0, a1=a1, n_real=n_real, contribs=contribs,
                pool_first=pool_first, pool_last=pool_last,
                contrib_idx=contrib_idx, NCONTRIB=NCONTRIB,
                cc_chunks=cc_chunks, cc_off=cc_off, cc_rows=cc_rows)
    return per_core, meta


# ---------------------------------------------------------------------------
# the Bass program (identical for all 8 cores; data differs per core)
# ---------------------------------------------------------------------------

def build_program(meta, DT=F32, CH=4):
    T, NA, N_pad = meta["T"], meta["NA"], meta["N_pad"]
    Epad, X_pad = meta["Epad"], meta["X_pad"]
    GRP, NGRP = meta["GRP"], meta["NGRP"]
    contribs = meta["contribs"]

    nc = bacc.Bacc("TRN2", target_bir_lowering=False, debug=False,
                   num_devices=C)
    DTZ = mybir.dt.float8e4  # Z-exchange dtype (halves collective traffic)

    xcatT0 = nc.dram_tensor("xcatT0", [128, Epad], DT, kind="ExternalInput")
    xcatT1 = nc.dram_tensor("xcatT1", [AF + BF - 128, Epad], DT,
                            kind="ExternalInput")
    xownT0 = nc.dram_tensor("xownT0", [128, N_pad], DT, kind="ExternalInput")
    xownT1 = nc.dram_tensor("xownT1", [AF - 127, N_pad], DT,
                            kind="ExternalInput")
    selm_in = nc.dram_tensor("selm_in", [128, meta["NCONTRIB"] * 128], DT,
                             kind="ExternalInput")
    selp_in = nc.dram_tensor("selp_in", [128, NA * 128], DT,
                             kind="ExternalInput")
    zrow_in = nc.dram_tensor("zrow_in", [128, T], I32, kind="ExternalInput")

    wname_shapes = dict(
        wix0=[128, H], wcat1=[AF + BF - 128, H],
        wh0=[128, H], wh1=[128, H], wh2=[H - 256, H],
        wox0=[128, H], wox1=[AF - 127, H],
        wom0=[128, H], wom1=[128, H], wom2=[H - 256, H],
    )
    w_in = {k: nc.dram_tensor(k, s, DT, kind="ExternalInput")
            for k, s in wname_shapes.items()}

    molp = nc.dram_tensor("molp", [NGRP * 128, H], F32, kind="ExternalOutput")

    zfull1 = nc.dram_tensor("zfull1", [C * N_pad, H], DTZ, addr_space="Shared")
    zfull2 = nc.dram_tensor("zfull2", [C * N_pad, H], DTZ, addr_space="Shared")
    cc_chunks = meta["cc_chunks"]

    HC = [(0, 128), (128, 256), (256, H)]  # hidden-dim K chunks
    XC = [(0, 128), (128, AF)]             # atom-feature K chunks

    with tile.TileContext(nc) as tc, ExitStack() as ctx:
        const = ctx.enter_context(tc.tile_pool(name="const", bufs=1))
        sb = ctx.enter_context(tc.tile_pool(name="sb", bufs=4))
        ps = ctx.enter_context(tc.tile_pool(name="ps", bufs=2, space="PSUM"))
        dram = ctx.enter_context(tc.tile_pool(name="dram", bufs=1,
                                              space="DRAM"))

        # ---- residents ----
        def cload(name, src, shape, dt):
            tl = const.tile(shape, dt, tag=name)
            nc.sync.dma_start(tl[:], src[:])
            return tl

        zrow_s = cload("zrow", zrow_in, [128, T], I32)
        w_s = {k: cload(k, w_in[k], wname_shapes[k], DT) for k in w_in}
        identF = const.tile([128, 128], F32, tag="identF")
        make_identity(nc, identF[:])


        msg1 = dram.tile([128, T * H], DT, tag="msg1")
        msg2 = dram.tile([128, T * H], DT, tag="msg2")
        zsh1 = dram.tile([N_pad, H], DTZ, tag="zsh1")
        zsh2 = dram.tile([N_pad, H], DTZ, tag="zsh2")

        psum_nei = {}
        psum_pool = {}

        def transpose_chunks(src_ap, chunks, ident):
            """PE-transpose column chunks of src_ap -> list of SBUF DT tiles.
            PSUM->SBUF casts alternate scalar/vector to balance engines."""
            out = []
            for ci, (c0, c1) in enumerate(chunks):
                w = c1 - c0
                tp = ps.tile([128, 128], F32, tag="pB", bufs=2)
                nc.tensor.transpose(tp[:w, :], src_ap[:, c0:c1], ident[:])
                trs = sb.tile([128, 128], DT, tag="trs")
                if ci == 1:
                    nc.scalar.copy(trs[:w, :], tp[:w, :])
                else:
                    nc.vector.tensor_copy(trs[:w, :], tp[:w, :])
                out.append(trs)
            return out

        cc_state = {}

        cur_zfull = [None]

        def drain_z(a, zsh):
            """psum_nei[a] -> Z = nei @ W_h -> zsh rows of ablock a."""
            nei_sb = sb.tile([128, H], F32, tag="neisb")
            nc.vector.tensor_copy(nei_sb[:], psum_nei.pop(a)[:])
            trs = transpose_chunks(nei_sb, HC, identF)
            zps = ps.tile([128, H], F32, tag="pD", bufs=1)
            for ci, ((c0, c1), wt) in enumerate(
                    zip(HC, (w_s["wh0"], w_s["wh1"], w_s["wh2"]))):
                w = c1 - c0
                nc.tensor.matmul(zps[:], lhsT=trs[ci][:w, :], rhs=wt[:],
                                 start=(ci == 0), stop=(ci == len(HC) - 1))
            zsb = sb.tile([128, H], DTZ, tag="zsb")
            nc.scalar.copy(zsb[:], zps[:])
            nc.sync.dma_start(zsh[a * 128:(a + 1) * 128, :], zsb[:])
            fire_cc(a, zsh, cur_zfull[0])

        def drain_final(a):
            """psum_nei[a] = atom_msg -> atom_h -> pool into group psum."""
            am_sb = sb.tile([128, H], F32, tag="neisb")
            nc.vector.tensor_copy(am_sb[:], psum_nei.pop(a)[:])
            hps = ps.tile([128, H], F32, tag="pA", bufs=2)
            at = transpose_chunks(am_sb, HC, identF)
            asl = slice(a * 128, (a + 1) * 128)
            xo0 = sb.tile([128, 128], DT, tag="xo0")
            nc.sync.dma_start(xo0[:], xownT0[:, asl])
            xo1 = sb.tile([AF - 127, 128], DT, tag="xo1")
            nc.sync.dma_start(xo1[:], xownT1[:, asl])
            parts = [(XC[0], xo0[:], w_s["wox0"]),
                     (XC[1], xo1[:], w_s["wox1"])] + \
                [((c0, c1), at[ci][:c1 - c0, :], w_s[k])
                 for ci, ((c0, c1), k) in enumerate(
                     zip(HC, ("wom0", "wom1", "wom2")))]
            for ci, ((c0, c1), lh, wt) in enumerate(parts):
                nc.tensor.matmul(hps[:], lhsT=lh, rhs=wt[:],
                                 start=(ci == 0), stop=(ci == len(parts) - 1))
            hrelu = sb.tile([128, H], DT, tag="hrelu")
            nc.scalar.activation(hrelu[:], hps[:], Relu)
            selp = sb.tile([128, 128], DT, tag="selp")
            nc.sync.dma_start(selp[:], selp_in[:, a * 128:(a + 1) * 128])
            j = a // GRP
            first = meta["pool_first"].get(a, False)
            last = meta["pool_last"].get(a, False)
            if first:
                psum_pool[j] = ps.tile([128, H], F32, tag="pD", name=f"pool{j}", bufs=1)
            nc.tensor.matmul(psum_pool[j][:], lhsT=selp[:], rhs=hrelu[:],
                             start=first, stop=last)
            if last:
                mol_sb = sb.tile([128, H], F32, tag="molsb")
                nc.vector.tensor_copy(mol_sb[:], psum_pool.pop(j)[:])
                nc.sync.dma_start(molp[j * 128:(j + 1) * 128, :], mol_sb[:])

        contrib_idx = meta["contrib_idx"]
        max_ncc = max(sum(len(contribs[t]) for t in range(t0, min(t0 + CH, T)))
                      for t0 in range(0, T, CH))

        def build_sels(t0, k):
            """Stream the host-precomputed sel matrices for tiles [t0,t0+k)."""
            idxs = [contrib_idx[(t, a)] for t in range(t0, t0 + k)
                    for (a, f, l) in contribs[t]]
            if not idxs:
                return None, 0
            i0, ncc = idxs[0], len(idxs)
            assert idxs == list(range(i0, i0 + ncc))
            selc = sb.tile([128, ncc * 128], DT, tag="sel",
                           padded_shape=[128, max_ncc * 128], name="selc")
            nc.sync.dma_start(selc[:], selm_in[:, i0 * 128:(i0 + ncc) * 128])
            return selc, i0

        def segsum_contrib(msg_ap, t, rnd, zsh, selc, i0):
            for (a, first, last) in contribs[t]:
                q = contrib_idx[(t, a)] - i0
                sel_ap = selc[:, q * 128:(q + 1) * 128]
                if first:
                    psum_nei[a] = ps.tile([128, H], F32, tag="pC", name=f"nei{a}", bufs=3)
                nc.tensor.matmul(psum_nei[a][:], lhsT=sel_ap, rhs=msg_ap,
                                 start=first, stop=last)
                if last:
                    if rnd < DEPTH:
                        drain_z(a, zsh)
                    else:
                        drain_final(a)

        # ---- stage A: initial messages + round-1 segsum ----
        def fire_cc(a, zsh, zfull):
            """After ablock a's drain DMA: if it completes a cc chunk, fire
            that chunk's AllGather."""
            st = cc_state.setdefault(id(zsh), dict(done=set()))
            st["done"].add(a)
            for (k0, k1) in cc_chunks:
                if all(x in st["done"] for x in range(k0, k1)) \
                        and (k0, k1) not in st.get("fired", set()):
                    st.setdefault("fired", set()).add((k0, k1))
                    ci = cc_chunks.index((k0, k1))
                    off = int(meta["cc_off"][ci])
                    nrows = int(meta["cc_rows"][ci])
                    cc = nc.gpsimd.collective_compute(
                        "AllGather", BYPASS,
                        replica_groups=[list(range(C))],
                        ins=[zsh[k0 * 128:k1 * 128, :]],
                        outs=[zfull[off:off + C * nrows, :]])
                    st["last_cc"] = cc

        # ---- stage A: msg1 = relu([x[src]||ea] @ W_i) + round-1 segsum ----
        # host supplies the per-slot concat table transposed; two matmuls
        # accumulate in PSUM and relu reads PSUM directly.
        cur_zfull[0] = zfull1
        for t0 in range(0, T, CH):
            k = min(CH, T - t0)
            selc, i0c = build_sels(t0, k)
            csl = slice(t0 * 128, (t0 + k) * 128)
            xc0 = sb.tile([128, k * 128], DT, tag="xc0", bufs=3)
            nc.sync.dma_start(xc0[:], xcatT0[:, csl])
            xc1 = sb.tile([AF + BF - 128, k * 128], DT, tag="xc1", bufs=3)
            nc.sync.dma_start(xc1[:], xcatT1[:, csl])
            msg_sb = sb.tile([128, k * H], DT, tag="msg", bufs=3)
            for j in range(k):
                t = t0 + j
                jsl = slice(j * 128, (j + 1) * 128)
                mps = ps.tile([128, H], F32, tag="pA", bufs=2)
                nc.tensor.matmul(mps[:], lhsT=xc0[:, jsl],
                                 rhs=w_s["wix0"][:], start=True, stop=False)
                nc.tensor.matmul(mps[:], lhsT=xc1[:, jsl],
                                 rhs=w_s["wcat1"][:], start=False, stop=True)
                m_ap = msg_sb[:, j * H:(j + 1) * H]
                nc.scalar.activation(m_ap, mps[:], Relu)
                segsum_contrib(m_ap, t, 1, zsh1, selc, i0c)
            nc.sync.dma_start(msg1[:, t0 * H:(t0 + k) * H], msg_sb[:])

        # ---- stages B (round 2) and C (round 3 + readout) ----
        def stage_mid(msg_in, msg_out, zfull, zsh, rnd, cc_prev):
            zflat = zfull[:]
            for t0 in range(0, T, CH):
                k = min(CH, T - t0)
                selc, i0c = build_sels(t0, k)
                ld = sb.tile([128, k * H], DT, tag="ld", bufs=3)
                nc.sync.dma_start(ld[:], msg_in[:, t0 * H:(t0 + k) * H])
                mrel = sb.tile([128, k * H], DT, tag="msg", bufs=3)
                zg = sb.tile([128, k * H], DTZ, tag="zg", bufs=3)
                for j in range(k):
                    t = t0 + j
                    nc.gpsimd.indirect_dma_start(
                        out=zg[:, j * H:(j + 1) * H], out_offset=None,
                        in_=zflat,
                        in_offset=IOA(ap=zrow_s[:, t:t + 1], axis=0))
                msum = sb.tile([128, k * H], DT, tag="msum")
                nc.vector.tensor_tensor(msum[:], ld[:], zg[:], op=ADD)
                nc.scalar.activation(mrel[:], msum[:], Relu)
                for j in range(k):
                    segsum_contrib(mrel[:, j * H:(j + 1) * H], t0 + j, rnd,
                                   zsh, selc, i0c)
                if msg_out is not None:
                    nc.sync.dma_start(msg_out[:, t0 * H:(t0 + k) * H],
                                      mrel[:])

        tc.strict_bb_all_engine_barrier()
        cur_zfull[0] = zfull2
        stage_mid(msg1, msg2, zfull1, zsh2, 2, None)
        tc.strict_bb_all_engine_barrier()
        stage_mid(msg2, None, zfull2, None, 3, None)

    nc.compile()
    return nc


# ---------------------------------------------------------------------------
# per-core input maps + output assembly
# ---------------------------------------------------------------------------

def np_dt(DT):
    import ml_dtypes
    return np.dtype(ml_dtypes.bfloat16) if DT == BF16 else np.float32


def make_in_maps(per_core, meta, W_i, W_h, W_o, b_o, DT=F32):
    T, NA = meta["T"], meta["NA"]
    d = np_dt(DT)
    weights = dict(
        wix0=W_i[:128], wcat1=W_i[128:],
        wh0=W_h[:128], wh1=W_h[128:256], wh2=W_h[256:],
        wox0=W_o[:128],
        wox1=np.concatenate([W_o[128:AF], b_o[None, :]], axis=0),
        wom0=W_o[AF:AF + 128], wom1=W_o[AF + 128:AF + 256],
        wom2=W_o[AF + 256:],
    )
    weights = {k: np.ascontiguousarray(v.astype(d)) for k, v in weights.items()}
    maps = []
    for pc in per_core:
        m = dict(weights)
        xcT = np.ascontiguousarray(pc["xcat"].T.astype(d))
        m["xcatT0"] = np.ascontiguousarray(xcT[:128])
        m["xcatT1"] = np.ascontiguousarray(xcT[128:])
        xoT = pc["x_own"].T.astype(d)  # [AF, N_pad]
        m["xownT0"] = np.ascontiguousarray(xoT[:128])
        m["xownT1"] = np.ascontiguousarray(np.concatenate(
            [xoT[128:], np.ones((1, xoT.shape[1]), dtype=d)], axis=0))
        m["selm_in"] = np.ascontiguousarray(pc["selm"].astype(d))
        m["selp_in"] = np.ascontiguousarray(pc["selp"].astype(d))
        m["zrow_in"] = np.ascontiguousarray(
            pc["zrow"].reshape(T, 128).T.astype(np.int32))
        maps.append(m)
    return maps


def assemble_mol(mol_parts, per_core, meta, num_graphs):
    out = np.zeros((num_graphs, H), dtype=np.float32)
    for c in range(C):
        g0s = per_core[c]["g0s"]
        for j in range(meta["NGRP"]):
            g0 = int(g0s[j])
            if g0 < 0:
                continue
            hi = min(g0 + 128, num_graphs)
            out[g0:hi] += mol_parts[c][j * 128: j * 128 + (hi - g0)]
    return out


# ---------------------------------------------------------------------------
# entry point
# ---------------------------------------------------------------------------

_prog_cache = {}


def _ensure_ntff_hook():
    """Register the axon NTFF profiling hook if the image's antenv lacks
    the axon_hooks module (profiling plumbing only; unused when
    trace=False)."""
    try:
        from antenv.axon_hooks import get_axon_ntff_profile_hook  # noqa
        return
    except ImportError:
        pass
    import types
    import antenv
    from trn_agent_boot.trn_boot import _ntff_profile_via_ctypes
    mod = types.ModuleType("antenv.axon_hooks")
    _h = [None]
    mod.set_axon_ntff_profile_hook = lambda h: _h.__setitem__(0, h)
    mod.get_axon_ntff_profile_hook = lambda: _h[0]
    sys.modules["antenv.axon_hooks"] = mod
    antenv.axon_hooks = mod
    try:
        hook = _ntff_profile_via_ctypes("/opt/axon/libaxon_pjrt.so")
        if hook is not None:
            mod.set_axon_ntff_profile_hook(hook)
    except Exception:
        pass
    # artifact upload needs a bucket; irrelevant for local profiling
    import concourse.bass_utils as _bu
    _bu.upload_artifacts = lambda tmpdir: tmpdir


def _run(inputs, DT=F32, trace=False, tmpdir=None):
    per_core, meta = preprocess(
        inputs["x"], inputs["edge_attr"], inputs["edge_index"],
        inputs["batch"], NUM_GRAPHS)
    key = (meta["T"], meta["NA"], meta["X_pad"], meta["NGRP"], str(DT),
           str(np.asarray(inputs["edge_index"])[:, 0]))
    ck = (meta["T"], meta["NA"], meta["X_pad"], meta["NGRP"], str(DT))
    if ck not in _prog_cache:
        _prog_cache[ck] = build_program(meta, DT=DT)
    nc = _prog_cache[ck]
    in_maps = make_in_maps(per_core, meta, inputs["W_i"], inputs["W_h"],
                           inputs["W_o"], inputs["b_o"], DT=DT)
    if trace:
        _ensure_ntff_hook()
    res = run_bass_kernel_spmd(nc, in_maps, list(range(C)), trace=trace,
                               tmpdir=tmpdir)
    mol_parts = [res.results[c]["molp"].astype(np.float32) for c in range(C)]
    out = assemble_mol(mol_parts, per_core, meta, NUM_GRAPHS)
    return out, res


def kernel(x, edge_attr, W_i, W_h, W_o, b_o, edge_index, batch):
    inputs = dict(x=np.asarray(x), edge_attr=np.asarray(edge_attr),
                  W_i=np.asarray(W_i), W_h=np.asarray(W_h),
                  W_o=np.asarray(W_o), b_o=np.asarray(b_o),
                  edge_index=np.asarray(edge_index),
                  batch=np.asarray(batch))
    out, _ = _run(inputs, DT=BF16)
    return out



# revision 27
# speedup vs baseline: 1.3820x; 1.0518x over previous
"""DMPNN encoder on 8 Trainium2 NeuronCores (Bass/Tile).

Strategy (data-parallel over graphs):
- Partition graphs into 8 contiguous chunks with ~equal atom counts
  (graph-aligned).  Each core owns the edges whose *target* atom lives in
  its chunk, sorted by local target -> segment-sum over targets is local.
- Per message-passing round, each core computes nei = segsum(msg) via
  selection-matrix matmuls, then Z = nei @ W_h on its own atoms, then the
  Z shards are AllGather'd across the 8 cores.  msg' = relu(msg + Z[src])
  only needs row gathers (indirect DMA) from the gathered Z table.
- Final round: atom_msg -> atom_h = relu([x||atom_msg] @ W_o + b_o) and
  sum-pool to graphs via selection matmuls; host sums partial group blocks.

All index manipulation is host-precomputed; the device does only dense
matmuls, elementwise ops, contiguous DMA and indirect row gathers.
"""

import os
import sys

for _p in ("/opt/trn_rl_repo", "/root/.axon_site/_ro/trn_rl_repo"):
    if os.path.isdir(_p) and _p not in sys.path:
        sys.path.insert(0, _p)

from contextlib import ExitStack

import numpy as np

import concourse.bass as bass
import concourse.tile as tile
from concourse import bacc, mybir
from concourse.bass_utils import run_bass_kernel_spmd
from concourse.masks import make_identity
from concourse.tile_rust import add_dep_helper

C = 8
H = 300
AF = 133
BF = 14
DEPTH = 3
NUM_GRAPHS = 4096

F32 = mybir.dt.float32
BF16 = mybir.dt.bfloat16
I32 = mybir.dt.int32

Relu = mybir.ActivationFunctionType.Relu
Copy = mybir.ActivationFunctionType.Copy
ADD = mybir.AluOpType.add
ISEQ = mybir.AluOpType.is_equal
BYPASS = mybir.AluOpType.bypass

IOA = bass.IndirectOffsetOnAxis


def ceil_to(x, m):
    return ((x + m - 1) // m) * m


# ---------------------------------------------------------------------------
# host-side preprocessing
# ---------------------------------------------------------------------------

def preprocess(x, edge_attr, edge_index, batch, num_graphs):
    N = x.shape[0]
    src = edge_index[0].astype(np.int64)
    tgt = edge_index[1].astype(np.int64)
    batch = batch.astype(np.int64)

    graph_start = np.searchsorted(batch, np.arange(num_graphs + 1))
    targets = (np.arange(1, C) * N) // C
    split_graphs = np.searchsorted(graph_start, targets)
    atom_splits = [0] + [int(graph_start[g]) for g in split_graphs] + [N]
    a0 = np.array(atom_splits[:-1])
    a1 = np.array(atom_splits[1:])
    n_real = a1 - a0
    N_pad = ceil_to(int(n_real.max()), 128) + 128
    NA = N_pad // 128

    owner = np.zeros(N, dtype=np.int64)
    loc = np.zeros(N, dtype=np.int64)
    for c in range(C):
        owner[a0[c]:a1[c]] = c
        loc[a0[c]:a1[c]] = np.arange(n_real[c])

    e_owner = owner[tgt]

    # per-core, per-ablock sorted edge lists (slot order within an ablock is
    # irrelevant for the selection matmuls)
    ab_eids = []  # [c][a] -> array of edge ids
    counts = np.zeros((C, NA), dtype=np.int64)
    for c in range(C):
        eids = np.nonzero(e_owner == c)[0]
        order = np.argsort(loc[tgt[eids]], kind="stable")
        eids = eids[order]
        ab = loc[tgt[eids]] // 128
        cuts = np.searchsorted(ab, np.arange(NA + 1))
        ab_eids.append([eids[cuts[a]:cuts[a + 1]] for a in range(NA)])
        counts[c] = cuts[1:] - cuts[:-1]

    for GRP in (64, 32, 16, 8, 4, 2, 1):
        ok = True
        for c in range(C):
            nb = int(n_real[c])
            for g0a in range(0, NA, GRP):
                alo, ahi = g0a * 128, min((g0a + GRP) * 128, nb)
                if alo >= nb:
                    continue
                if batch[a0[c] + ahi - 1] - batch[a0[c] + alo] >= 128:
                    ok = False
                    break
            if not ok:
                break
        if ok:
            break
    NGRP = (NA + GRP - 1) // GRP

    # greedy segmentation over ablocks: keep the union live-depth of nei
    # accumulators <= DMAX so only DMAX PSUM slots are needed
    DMAX = 3

    def seg_stats(s, e):
        P = np.zeros((C,), dtype=np.int64)
        first = {}
        last = {}
        for a in range(s, e):
            cnt = counts[:, a]
            act = cnt > 0
            if act.any():
                first[a] = int((P[act] // 128).min())
                last[a] = int(((P[act] + cnt[act] - 1) // 128).max())
            P += cnt
        ntile = int(-(-P.max() // 128)) if P.max() else 1
        depth = 0
        for t in range(ntile):
            depth = max(depth, sum(1 for a in first
                                   if first[a] <= t <= last[a]))
        return first, last, ntile, depth

    segments = []  # (a_start, a_end, first, last, ntiles)
    s = 0
    while s < NA:
        e = s + 1
        stats = seg_stats(s, e)
        while e < NA and (e % GRP) != 0:
            cand = seg_stats(s, e + 1)
            if cand[3] > DMAX:
                break
            e += 1
            stats = cand
        segments.append((s, e, stats[0], stats[1], stats[2]))
        s = e

    T = sum(sg[4] for sg in segments)
    Epad = T * 128

    contribs = [[] for _ in range(T)]
    base = 0
    for (s, e, first, last, ntiles) in segments:
        for a in range(s, e):
            if a in first:
                ts = range(base + first[a], base + last[a] + 1)
            else:
                ts = [base]  # no edges anywhere: zero matmul, still drains
            ts = list(ts)
            for i, t in enumerate(ts):
                contribs[t].append((a, i == 0, i == len(ts) - 1))
        base += ntiles

    # contrib enumeration order + chunk boundaries for batched sel builds
    contrib_idx = {}
    ci = 0
    for t in range(T):
        for (a, f, l) in contribs[t]:
            contrib_idx[(t, a)] = ci
            ci += 1
    NCONTRIB = ci

    drain_seq = [a for t in range(T) for (a, f, l) in contribs[t] if l]
    pool_first = {}
    pool_last = {}
    seen = {}
    for a in drain_seq:
        j = a // GRP
        if j not in seen:
            pool_first[a] = True
        seen[j] = a
    for j, a in seen.items():
        pool_last[a] = True

    # collective chunking: chunk-major Z-table layout so each chunk's
    # AllGather output is a contiguous region.  Sizes decrease so the last
    # chunk's exchange latency (the only un-overlappable part) is tiny.
    gu = (NA + GRP - 1) // GRP  # group units
    unit_counts = []
    rem = gu
    for frac in (6, 6, 7, 8, 9, 10, 12, 14, 16, 16):
        take = min(max(1, gu // frac), rem)
        if take:
            unit_counts.append(take)
            rem -= take
        if rem == 0:
            break
    while rem > 0:
        unit_counts.append(1)
        rem -= 1
    cc_chunks = []
    a = 0
    for u in unit_counts:
        e = min(a + u * GRP, NA)
        cc_chunks.append((a, e))
        a = e
        if a >= NA:
            break
    n_cc = len(cc_chunks)
    cc_start = np.array([k0 * 128 for (k0, k1) in cc_chunks])
    cc_rows = np.array([(k1 - k0) * 128 for (k0, k1) in cc_chunks])
    cc_off = np.concatenate([[0], np.cumsum(C * cc_rows)])[:-1]

    # lay out slots: per core, segments concatenated, padded to common size
    per_core = []
    for c in range(C):
        slot_eid = np.full(Epad, -1, dtype=np.int64)
        base = 0
        for (s, e, first, last, ntiles) in segments:
            seg = np.concatenate([ab_eids[c][a] for a in range(s, e)]) \
                if e > s else np.zeros(0, np.int64)
            slot_eid[base:base + len(seg)] = seg
            base += ntiles * 128
        real = slot_eid >= 0
        eids = slot_eid[real]
        ne = int(real.sum())
        tloc = np.full(Epad, -1, dtype=np.int64)
        tloc[real] = loc[tgt[eids]]
        src_g = np.zeros(Epad, dtype=np.int64)
        src_g[real] = src[eids]
        ea = np.zeros((Epad, BF), dtype=np.float32)
        ea[real] = edge_attr[eids]
        zrow = np.zeros(Epad, dtype=np.int64)
        ol = owner[src_g[real]]
        ll = loc[src_g[real]]
        ck = np.searchsorted(cc_start, ll, side="right") - 1
        zrow[real] = cc_off[ck] + ol * cc_rows[ck] + (ll - cc_start[ck])
        uniq, inv = np.unique(src_g[real], return_inverse=True)
        xidx = np.zeros(Epad, dtype=np.int64)
        xidx[real] = inv
        xcat = np.zeros((Epad, AF + BF), dtype=np.float32)
        xcat[:, :AF] = x[src_g]
        xcat[:, AF:] = ea
        ctabm = np.zeros((128, NCONTRIB), dtype=np.float32)
        for t in range(T):
            for (a, f, l) in contribs[t]:
                i = contrib_idx[(t, a)]
                ctabm[:, i] = (tloc[t * 128:(t + 1) * 128] - 128 * a)
        selm = (ctabm[:, :, None] == np.arange(128, dtype=np.float32)
                ).astype(np.float32).reshape(128, NCONTRIB * 128)
        per_core.append(dict(ne=ne, real=real, tloc=tloc, ea=ea, zrow=zrow,
                             xidx=xidx, uniq=uniq, n_uniq=len(uniq),
                             selm=selm, xcat=xcat))

    X_pad = max(pc["n_uniq"] for pc in per_core) + 1
    for pc in per_core:
        pc["xidx"][~pc["real"]] = X_pad - 1
        xs = np.zeros((X_pad, AF), dtype=np.float32)
        xs[: pc["n_uniq"]] = x[pc["uniq"]]
        pc["x_sub"] = xs

    for c in range(C):
        gloc = np.full(N_pad, 1 << 20, dtype=np.int64)
        g0s = np.full(NGRP, -1, dtype=np.int64)
        nb = int(n_real[c])
        for j in range(NGRP):
            alo = j * GRP * 128
            ahi = min(alo + GRP * 128, nb)
            if alo >= nb:
                continue
            g0 = batch[a0[c] + alo]
            g0s[j] = g0
            gloc[alo:ahi] = batch[a0[c] + alo: a0[c] + ahi] - g0
        per_core[c]["g0s"] = g0s
        glocm = np.minimum(gloc, 1 << 20).reshape(NA, 128).T.astype(np.float32)
        per_core[c]["selp"] = (
            glocm[:, :, None] == np.arange(128, dtype=np.float32)
        ).astype(np.float32).reshape(128, NA * 128)
        xo = np.zeros((N_pad, AF), dtype=np.float32)
        xo[:nb] = x[a0[c]: a1[c]]
        per_core[c]["x_own"] = xo

    meta = dict(N_pad=N_pad, NA=NA, Epad=Epad, T=T, X_pad=X_pad, GRP=GRP,
                NGRP=NGRP, a0=a0, a1=a1, n_real=n_real, contribs=contribs,
                pool_first=pool_first, pool_last=pool_last,
                contrib_idx=contrib_idx, NCONTRIB=NCONTRIB,
                cc_chunks=cc_chunks, cc_off=cc_off, cc_rows=cc_rows)
    return per_core, meta


# ---------------------------------------------------------------------------
# the Bass program (identical for all 8 cores; data differs per core)
# ---------------------------------------------------------------------------

def build_program(meta, DT=F32, CH=4):
    T, NA, N_pad = meta["T"], meta["NA"], meta["N_pad"]
    Epad, X_pad = meta["Epad"], meta["X_pad"]
    GRP, NGRP = meta["GRP"], meta["NGRP"]
    contribs = meta["contribs"]

    nc = bacc.Bacc("TRN2", target_bir_lowering=False, debug=False,
                   num_devices=C)
    DTZ = mybir.dt.float8e4  # Z-exchange dtype (halves collective traffic)

    xcatT0 = nc.dram_tensor("xcatT0", [128, Epad], DT, kind="ExternalInput")
    xcatT1 = nc.dram_tensor("xcatT1", [AF + BF - 128, Epad], DT,
                            kind="ExternalInput")
    xownT0 = nc.dram_tensor("xownT0", [128, N_pad], DT, kind="ExternalInput")
    xownT1 = nc.dram_tensor("xownT1", [AF - 127, N_pad], DT,
                            kind="ExternalInput")
    selm_in = nc.dram_tensor("selm_in", [128, meta["NCONTRIB"] * 128], DT,
                             kind="ExternalInput")
    selp_in = nc.dram_tensor("selp_in", [128, NA * 128], DT,
                             kind="ExternalInput")
    zrow_in = nc.dram_tensor("zrow_in", [128, T], I32, kind="ExternalInput")

    wname_shapes = dict(
        wix0=[128, H], wcat1=[AF + BF - 128, H],
        wh0=[128, H], wh1=[128, H], wh2=[H - 256, H],
        wox0=[128, H], wox1=[AF - 127, H],
        wom0=[128, H], wom1=[128, H], wom2=[H - 256, H],
    )
    w_in = {k: nc.dram_tensor(k, s, DT, kind="ExternalInput")
            for k, s in wname_shapes.items()}

    molp = nc.dram_tensor("molp", [NGRP * 128, H], F32, kind="ExternalOutput")

    zfull1 = nc.dram_tensor("zfull1", [C * N_pad, H], DTZ, addr_space="Shared")
    zfull2 = nc.dram_tensor("zfull2", [C * N_pad, H], DTZ, addr_space="Shared")
    cc_chunks = meta["cc_chunks"]

    HC = [(0, 128), (128, 256), (256, H)]  # hidden-dim K chunks
    XC = [(0, 128), (128, AF)]             # atom-feature K chunks

    with tile.TileContext(nc) as tc, ExitStack() as ctx:
        const = ctx.enter_context(tc.tile_pool(name="const", bufs=1))
        sb = ctx.enter_context(tc.tile_pool(name="sb", bufs=4))
        ps = ctx.enter_context(tc.tile_pool(name="ps", bufs=2, space="PSUM"))
        dram = ctx.enter_context(tc.tile_pool(name="dram", bufs=1,
                                              space="DRAM"))

        # ---- residents ----
        def cload(name, src, shape, dt):
            tl = const.tile(shape, dt, tag=name)
            nc.sync.dma_start(tl[:], src[:])
            return tl

        zrow_s = cload("zrow", zrow_in, [128, T], I32)
        w_s = {k: cload(k, w_in[k], wname_shapes[k], DT) for k in w_in}
        identF = const.tile([128, 128], F32, tag="identF")
        make_identity(nc, identF[:])


        msg1 = dram.tile([128, T * H], DTZ, tag="msg1")
        msg2 = dram.tile([128, T * H], DTZ, tag="msg2")
        zsh1 = dram.tile([N_pad, H], DTZ, tag="zsh1")
        zsh2 = dram.tile([N_pad, H], DTZ, tag="zsh2")

        psum_nei = {}
        psum_pool = {}

        def transpose_chunks(src_ap, chunks, ident):
            """PE-transpose column chunks of src_ap -> list of SBUF DT tiles.
            PSUM->SBUF casts alternate scalar/vector to balance engines."""
            out = []
            for ci, (c0, c1) in enumerate(chunks):
                w = c1 - c0
                tp = ps.tile([128, 128], F32, tag="pB", bufs=2)
                nc.tensor.transpose(tp[:w, :], src_ap[:, c0:c1], ident[:])
                trs = sb.tile([128, 128], DT, tag="trs")
                if ci == 1:
                    nc.scalar.copy(trs[:w, :], tp[:w, :])
                else:
                    nc.vector.tensor_copy(trs[:w, :], tp[:w, :])
                out.append(trs)
            return out

        cc_state = {}

        cur_zfull = [None]

        def drain_z(a, zsh):
            """psum_nei[a] -> Z = nei @ W_h -> zsh rows of ablock a."""
            nei_sb = sb.tile([128, H], F32, tag="neisb")
            nc.vector.tensor_copy(nei_sb[:], psum_nei.pop(a)[:])
            trs = transpose_chunks(nei_sb, HC, identF)
            zps = ps.tile([128, H], F32, tag="pD", bufs=1)
            for ci, ((c0, c1), wt) in enumerate(
                    zip(HC, (w_s["wh0"], w_s["wh1"], w_s["wh2"]))):
                w = c1 - c0
                nc.tensor.matmul(zps[:], lhsT=trs[ci][:w, :], rhs=wt[:],
                                 start=(ci == 0), stop=(ci == len(HC) - 1))
            zsb = sb.tile([128, H], DTZ, tag="zsb")
            nc.scalar.copy(zsb[:], zps[:])
            nc.sync.dma_start(zsh[a * 128:(a + 1) * 128, :], zsb[:])
            fire_cc(a, zsh, cur_zfull[0])

        def drain_final(a):
            """psum_nei[a] = atom_msg -> atom_h -> pool into group psum."""
            am_sb = sb.tile([128, H], F32, tag="neisb")
            nc.vector.tensor_copy(am_sb[:], psum_nei.pop(a)[:])
            hps = ps.tile([128, H], F32, tag="pA", bufs=2)
            at = transpose_chunks(am_sb, HC, identF)
            asl = slice(a * 128, (a + 1) * 128)
            xo0 = sb.tile([128, 128], DT, tag="xo0")
            nc.sync.dma_start(xo0[:], xownT0[:, asl])
            xo1 = sb.tile([AF - 127, 128], DT, tag="xo1")
            nc.sync.dma_start(xo1[:], xownT1[:, asl])
            parts = [(XC[0], xo0[:], w_s["wox0"]),
                     (XC[1], xo1[:], w_s["wox1"])] + \
                [((c0, c1), at[ci][:c1 - c0, :], w_s[k])
                 for ci, ((c0, c1), k) in enumerate(
                     zip(HC, ("wom0", "wom1", "wom2")))]
            for ci, ((c0, c1), lh, wt) in enumerate(parts):
                nc.tensor.matmul(hps[:], lhsT=lh, rhs=wt[:],
                                 start=(ci == 0), stop=(ci == len(parts) - 1))
            hrelu = sb.tile([128, H], DT, tag="hrelu")
            nc.scalar.activation(hrelu[:], hps[:], Relu)
            selp = sb.tile([128, 128], DT, tag="selp")
            nc.sync.dma_start(selp[:], selp_in[:, a * 128:(a + 1) * 128])
            j = a // GRP
            first = meta["pool_first"].get(a, False)
            last = meta["pool_last"].get(a, False)
            if first:
                psum_pool[j] = ps.tile([128, H], F32, tag="pD", name=f"pool{j}", bufs=1)
            nc.tensor.matmul(psum_pool[j][:], lhsT=selp[:], rhs=hrelu[:],
                             start=first, stop=last)
            if last:
                mol_sb = sb.tile([128, H], F32, tag="molsb")
                nc.vector.tensor_copy(mol_sb[:], psum_pool.pop(j)[:])
                nc.sync.dma_start(molp[j * 128:(j + 1) * 128, :], mol_sb[:])

        contrib_idx = meta["contrib_idx"]
        max_ncc = max(sum(len(contribs[t]) for t in range(t0, min(t0 + CH, T)))
                      for t0 in range(0, T, CH))

        def build_sels(t0, k):
            """Stream the host-precomputed sel matrices for tiles [t0,t0+k)."""
            idxs = [contrib_idx[(t, a)] for t in range(t0, t0 + k)
                    for (a, f, l) in contribs[t]]
            if not idxs:
                return None, 0
            i0, ncc = idxs[0], len(idxs)
            assert idxs == list(range(i0, i0 + ncc))
            selc = sb.tile([128, ncc * 128], DT, tag="sel",
                           padded_shape=[128, max_ncc * 128], name="selc")
            nc.sync.dma_start(selc[:], selm_in[:, i0 * 128:(i0 + ncc) * 128])
            return selc, i0

        def segsum_contrib(msg_ap, t, rnd, zsh, selc, i0):
            for (a, first, last) in contribs[t]:
                q = contrib_idx[(t, a)] - i0
                sel_ap = selc[:, q * 128:(q + 1) * 128]
                if first:
                    psum_nei[a] = ps.tile([128, H], F32, tag="pC", name=f"nei{a}", bufs=3)
                nc.tensor.matmul(psum_nei[a][:], lhsT=sel_ap, rhs=msg_ap,
                                 start=first, stop=last)
                if last:
                    if rnd < DEPTH:
                        drain_z(a, zsh)
                    else:
                        drain_final(a)

        # ---- stage A: initial messages + round-1 segsum ----
        def fire_cc(a, zsh, zfull):
            """After ablock a's drain DMA: if it completes a cc chunk, fire
            that chunk's AllGather."""
            st = cc_state.setdefault(id(zsh), dict(done=set()))
            st["done"].add(a)
            for (k0, k1) in cc_chunks:
                if all(x in st["done"] for x in range(k0, k1)) \
                        and (k0, k1) not in st.get("fired", set()):
                    st.setdefault("fired", set()).add((k0, k1))
                    ci = cc_chunks.index((k0, k1))
                    off = int(meta["cc_off"][ci])
                    nrows = int(meta["cc_rows"][ci])
                    cc = nc.gpsimd.collective_compute(
                        "AllGather", BYPASS,
                        replica_groups=[list(range(C))],
                        ins=[zsh[k0 * 128:k1 * 128, :]],
                        outs=[zfull[off:off + C * nrows, :]])
                    st["last_cc"] = cc

        # ---- stage A: msg1 = relu([x[src]||ea] @ W_i) + round-1 segsum ----
        # host supplies the per-slot concat table transposed; two matmuls
        # accumulate in PSUM and relu reads PSUM directly.
        cur_zfull[0] = zfull1
        for t0 in range(0, T, CH):
            k = min(CH, T - t0)
            selc, i0c = build_sels(t0, k)
            csl = slice(t0 * 128, (t0 + k) * 128)
            xc0 = sb.tile([128, k * 128], DT, tag="xc0", bufs=3)
            nc.sync.dma_start(xc0[:], xcatT0[:, csl])
            xc1 = sb.tile([AF + BF - 128, k * 128], DT, tag="xc1", bufs=3)
            nc.sync.dma_start(xc1[:], xcatT1[:, csl])
            msg_sb = sb.tile([128, k * H], DT, tag="msg", bufs=3)
            for j in range(k):
                t = t0 + j
                jsl = slice(j * 128, (j + 1) * 128)
                mps = ps.tile([128, H], F32, tag="pA", bufs=2)
                nc.tensor.matmul(mps[:], lhsT=xc0[:, jsl],
                                 rhs=w_s["wix0"][:], start=True, stop=False)
                nc.tensor.matmul(mps[:], lhsT=xc1[:, jsl],
                                 rhs=w_s["wcat1"][:], start=False, stop=True)
                m_ap = msg_sb[:, j * H:(j + 1) * H]
                nc.scalar.activation(m_ap, mps[:], Relu)
                segsum_contrib(m_ap, t, 1, zsh1, selc, i0c)
            msg8 = sb.tile([128, k * H], DTZ, tag="msg8", bufs=3)
            nc.vector.tensor_copy(msg8[:], msg_sb[:])
            nc.sync.dma_start(msg1[:, t0 * H:(t0 + k) * H], msg8[:])

        # ---- stages B (round 2) and C (round 3 + readout) ----
        def stage_mid(msg_in, msg_out, zfull, zsh, rnd, cc_prev):
            zflat = zfull[:]
            for t0 in range(0, T, CH):
                k = min(CH, T - t0)
                selc, i0c = build_sels(t0, k)
                ld = sb.tile([128, k * H], DTZ, tag="ld", bufs=3)
                nc.sync.dma_start(ld[:], msg_in[:, t0 * H:(t0 + k) * H])
                mrel = sb.tile([128, k * H], DT, tag="msg", bufs=3)
                zg = sb.tile([128, k * H], DTZ, tag="zg", bufs=3)
                for j in range(k):
                    t = t0 + j
                    nc.gpsimd.indirect_dma_start(
                        out=zg[:, j * H:(j + 1) * H], out_offset=None,
                        in_=zflat,
                        in_offset=IOA(ap=zrow_s[:, t:t + 1], axis=0))
                msum = sb.tile([128, k * H], DT, tag="msum")
                nc.vector.tensor_tensor(msum[:], ld[:], zg[:], op=ADD)
                nc.scalar.activation(mrel[:], msum[:], Relu)
                for j in range(k):
                    segsum_contrib(mrel[:, j * H:(j + 1) * H], t0 + j, rnd,
                                   zsh, selc, i0c)
                if msg_out is not None:
                    msg8 = sb.tile([128, k * H], DTZ, tag="msg8", bufs=3)
                    nc.vector.tensor_copy(msg8[:], mrel[:])
                    nc.sync.dma_start(msg_out[:, t0 * H:(t0 + k) * H],
                                      msg8[:])

        tc.strict_bb_all_engine_barrier()
        cur_zfull[0] = zfull2
        stage_mid(msg1, msg2, zfull1, zsh2, 2, None)
        tc.strict_bb_all_engine_barrier()
        stage_mid(msg2, None, zfull2, None, 3, None)

    nc.compile()
    return nc


# ---------------------------------------------------------------------------
# per-core input maps + output assembly
# ---------------------------------------------------------------------------

def np_dt(DT):
    import ml_dtypes
    return np.dtype(ml_dtypes.bfloat16) if DT == BF16 else np.float32


def make_in_maps(per_core, meta, W_i, W_h, W_o, b_o, DT=F32):
    T, NA = meta["T"], meta["NA"]
    d = np_dt(DT)
    weights = dict(
        wix0=W_i[:128], wcat1=W_i[128:],
        wh0=W_h[:128], wh1=W_h[128:256], wh2=W_h[256:],
        wox0=W_o[:128],
        wox1=np.concatenate([W_o[128:AF], b_o[None, :]], axis=0),
        wom0=W_o[AF:AF + 128], wom1=W_o[AF + 128:AF + 256],
        wom2=W_o[AF + 256:],
    )
    weights = {k: np.ascontiguousarray(v.astype(d)) for k, v in weights.items()}
    maps = []
    for pc in per_core:
        m = dict(weights)
        xcT = np.ascontiguousarray(pc["xcat"].T.astype(d))
        m["xcatT0"] = np.ascontiguousarray(xcT[:128])
        m["xcatT1"] = np.ascontiguousarray(xcT[128:])
        xoT = pc["x_own"].T.astype(d)  # [AF, N_pad]
        m["xownT0"] = np.ascontiguousarray(xoT[:128])
        m["xownT1"] = np.ascontiguousarray(np.concatenate(
            [xoT[128:], np.ones((1, xoT.shape[1]), dtype=d)], axis=0))
        m["selm_in"] = np.ascontiguousarray(pc["selm"].astype(d))
        m["selp_in"] = np.ascontiguousarray(pc["selp"].astype(d))
        m["zrow_in"] = np.ascontiguousarray(
            pc["zrow"].reshape(T, 128).T.astype(np.int32))
        maps.append(m)
    return maps


def assemble_mol(mol_parts, per_core, meta, num_graphs):
    out = np.zeros((num_graphs, H), dtype=np.float32)
    for c in range(C):
        g0s = per_core[c]["g0s"]
        for j in range(meta["NGRP"]):
            g0 = int(g0s[j])
            if g0 < 0:
                continue
            hi = min(g0 + 128, num_graphs)
            out[g0:hi] += mol_parts[c][j * 128: j * 128 + (hi - g0)]
    return out


# ---------------------------------------------------------------------------
# entry point
# ---------------------------------------------------------------------------

_prog_cache = {}


def _ensure_ntff_hook():
    """Register the axon NTFF profiling hook if the image's antenv lacks
    the axon_hooks module (profiling plumbing only; unused when
    trace=False)."""
    try:
        from antenv.axon_hooks import get_axon_ntff_profile_hook  # noqa
        return
    except ImportError:
        pass
    import types
    import antenv
    from trn_agent_boot.trn_boot import _ntff_profile_via_ctypes
    mod = types.ModuleType("antenv.axon_hooks")
    _h = [None]
    mod.set_axon_ntff_profile_hook = lambda h: _h.__setitem__(0, h)
    mod.get_axon_ntff_profile_hook = lambda: _h[0]
    sys.modules["antenv.axon_hooks"] = mod
    antenv.axon_hooks = mod
    try:
        hook = _ntff_profile_via_ctypes("/opt/axon/libaxon_pjrt.so")
        if hook is not None:
            mod.set_axon_ntff_profile_hook(hook)
    except Exception:
        pass
    # artifact upload needs a bucket; irrelevant for local profiling
    import concourse.bass_utils as _bu
    _bu.upload_artifacts = lambda tmpdir: tmpdir


def _run(inputs, DT=F32, trace=False, tmpdir=None):
    per_core, meta = preprocess(
        inputs["x"], inputs["edge_attr"], inputs["edge_index"],
        inputs["batch"], NUM_GRAPHS)
    key = (meta["T"], meta["NA"], meta["X_pad"], meta["NGRP"], str(DT),
           str(np.asarray(inputs["edge_index"])[:, 0]))
    ck = (meta["T"], meta["NA"], meta["X_pad"], meta["NGRP"], str(DT))
    if ck not in _prog_cache:
        _prog_cache[ck] = build_program(meta, DT=DT)
    nc = _prog_cache[ck]
    in_maps = make_in_maps(per_core, meta, inputs["W_i"], inputs["W_h"],
                           inputs["W_o"], inputs["b_o"], DT=DT)
    if trace:
        _ensure_ntff_hook()
    res = run_bass_kernel_spmd(nc, in_maps, list(range(C)), trace=trace,
                               tmpdir=tmpdir)
    mol_parts = [res.results[c]["molp"].astype(np.float32) for c in range(C)]
    out = assemble_mol(mol_parts, per_core, meta, NUM_GRAPHS)
    return out, res


def kernel(x, edge_attr, W_i, W_h, W_o, b_o, edge_index, batch):
    inputs = dict(x=np.asarray(x), edge_attr=np.asarray(edge_attr),
                  W_i=np.asarray(W_i), W_h=np.asarray(W_h),
                  W_o=np.asarray(W_o), b_o=np.asarray(b_o),
                  edge_index=np.asarray(edge_index),
                  batch=np.asarray(batch))
    out, _ = _run(inputs, DT=BF16)
    return out

